# revision 15
# baseline (speedup 1.0000x reference)
"""Causal self-attention with sink logit on 8 Trainium2 NeuronCores.

nn_CausalSelfAttention: B=2, T=2048, C=1024, H=16, D=64.
    qkv = x @ w_qkv; per-head causal attention with a per-head sink logit in
    the softmax denominator; out = y @ w_proj.

Sharding: 8 cores = 2 batches x 4 head-groups (data-parallel over B,
tensor-parallel over heads). Each core computes its batch's qkv projection
restricted to its 4 heads, flash-style causal attention (S^T layout,
denominator via an appended ones-block in the V matmul), and the partial
output projection against its w_proj row-slice. All matmul operands are
bf16 (1 cycle/row on the PE vs 2 for fp32r; fp32 PSUM accumulation), the
diagonal blocks compute only the un-masked column suffix, the per-head sink
is added to the denominator on the vector engine right before the
reciprocal (no seed matmuls), and 1/denom uses the fast approximate DVE
reciprocal. The host sums the 4 per-head-group bf16 partials per batch in
float64 (the "all-reduce after c_proj", done host-side since the full
output is assembled host-side anyway).

kernel(**inputs) takes the FULL unsharded inputs and returns the FULL output.
"""
from contextlib import ExitStack

import numpy as np
import ml_dtypes

F32 = None
BF16 = None

P_ = 128          # partitions
QB = 512          # psum bank width (fp32)
D = 64            # head dim
HPC = 4           # heads per core
NPAIR = 2
B, T, C, H = 2, 2048, 1024, 16
N_CORES = 8


def _bf16(x):
    return np.ascontiguousarray(np.asarray(x, dtype=np.float32)).astype(
        ml_dtypes.bfloat16)


# feature flags (bisection)
TRIM_S = True     # compute only un-masked suffix of diagonal S blocks
TRIM_AV = True    # narrow AV matmuls on diagonal blocks
SINK_ADD = True    # sink via tensor_scalar_add before recip (else seed matmul)
FAST_RECIP = True  # reciprocal_approx_fast instead of reciprocal
GPSIMD_MASK = False  # causal mask mult on gpsimd (else vector)


def _build_bass():
    import concourse.mybir as mybir
    import concourse.tile as tile
    from concourse import bacc

    global F32, BF16
    F32 = mybir.dt.float32
    BF16 = mybir.dt.bfloat16

    CCH = C // P_             # C chunks
    GW = min(QB, T // 2)      # q/t group width
    NG = T // GW              # groups
    NTCG = GW // P_           # t-chunks per group
    scale = 1.0 / np.sqrt(D)

    nc = bacc.Bacc("TRN2", target_bir_lowering=False, debug=False,
                   num_devices=N_CORES)

    xt_d = nc.dram_tensor("xt", [C, T], BF16, kind="ExternalInput")
    wqk_d = nc.dram_tensor("wqk", [C, 2 * HPC * D], BF16, kind="ExternalInput")
    wv_d = nc.dram_tensor("wv", [C, HPC * D], BF16, kind="ExternalInput")
    wproj_d = nc.dram_tensor("wproj", [HPC * D, C], BF16, kind="ExternalInput")
    es_d = nc.dram_tensor("essc", [P_, HPC], F32, kind="ExternalInput")
    esr_d = nc.dram_tensor("esrows", [1, HPC * P_], BF16, kind="ExternalInput")
    ones_d = nc.dram_tensor("ones512", [1, QB], BF16, kind="ExternalInput")
    onesc_d = nc.dram_tensor("onesc", [P_, D], BF16, kind="ExternalInput")
    mask_d = nc.dram_tensor("mask", [P_, P_], BF16, kind="ExternalInput")
    masks4_d = nc.dram_tensor("masks4", [4, P_, QB], BF16, kind="ExternalInput")
    out_d = nc.dram_tensor("out", [T, C], BF16, kind="ExternalOutput")

    with tile.TileContext(nc) as tc, ExitStack() as ctx:
        pool = ctx.enter_context(tc.tile_pool(name="pool", bufs=1))
        xt_pool = ctx.enter_context(tc.tile_pool(name="xt", bufs=2 * CCH + 2))
        work = ctx.enter_context(tc.tile_pool(name="work", bufs=2))
        psum = ctx.enter_context(tc.tile_pool(name="ps", bufs=1, space="PSUM"))

        es = pool.tile([P_, HPC], F32, tag="es")
        esr = pool.tile([1, HPC * P_], BF16, tag="esr")
        ones = pool.tile([1, QB], BF16, tag="ones")
        onesc = pool.tile([P_, D], BF16, tag="onesc")
        maskv = pool.tile([P_, P_], BF16, tag="maskv")
        mask4 = pool.tile([P_, 4, QB], BF16, tag="mask4")
        nc.sync.dma_start(es[:], es_d.ap())
        nc.sync.dma_start(esr[:], esr_d.ap())
        nc.sync.dma_start(ones[:], ones_d.ap())
        nc.sync.dma_start(onesc[:], onesc_d.ap())
        nc.sync.dma_start(maskv[:], mask_d.ap())
        nc.sync.dma_start(mask4[:], masks4_d.ap().rearrange("v p q -> p v q"))

        wqk = pool.tile([P_, CCH, 2 * HPC * D], BF16, tag="wqk")
        wv = pool.tile([P_, CCH, HPC * D], BF16, tag="wv")
        wproj = pool.tile([P_, 2, C], BF16, tag="wproj")
        for c in range(CCH):
            nc.sync.dma_start(wqk[:, c, :], wqk_d.ap()[c * P_:(c + 1) * P_, :])
            nc.sync.dma_start(wv[:, c, :], wv_d.ap()[c * P_:(c + 1) * P_, :])
        nc.sync.dma_start(wproj[:], wproj_d.ap().rearrange("(co ci) m -> ci co m", ci=P_))

        QKT = pool.tile([P_, 2 * NPAIR, T], BF16, tag="qkt")
        VO = pool.tile([P_, T // P_, HPC, P_], BF16, tag="vo")
        YT = pool.tile([P_, NPAIR, T], BF16, tag="yt")

        nc.vector.tensor_copy(
            VO[:, :, :, D:P_],
            onesc[:, None, None, :].to_broadcast([P_, T // P_, HPC, D]))
        for g in range(NG):
            tg0 = g * GW
            xg = [xt_pool.tile([P_, GW], BF16, tag="xt", name=f"x{g}_{c}")
                  for c in range(CCH)]
            for c in range(CCH):
                nc.scalar.dma_start(xg[c][:], xt_d.ap()[c * P_:(c + 1) * P_,
                                                        tg0:tg0 + GW])
            for m in range(2 * NPAIR):
                ps = psum.tile([P_, GW], F32, tag="qk", bufs=2, name=f"qk{g}_{m}")
                for c in range(CCH):
                    nc.tensor.matmul(
                        ps[:], wqk[:, c, m * P_:(m + 1) * P_], xg[c][:],
                        start=(c == 0), stop=(c == CCH - 1))
                nc.vector.tensor_copy(QKT[:, m, tg0:tg0 + GW], ps[:])
            for tcl in range(NTCG):
                tc_g = g * NTCG + tcl
                ps = psum.tile([P_, HPC * D], F32, tag="qk", bufs=2,
                               name=f"vps{g}_{tcl}")
                for c in range(CCH):
                    nc.tensor.matmul(
                        ps[:], xg[c][:, tcl * P_:(tcl + 1) * P_], wv[:, c, :],
                        start=(c == 0), stop=(c == CCH - 1))
                nc.vector.tensor_copy(
                    VO[:, tc_g, :, 0:D],
                    ps[:].rearrange("p (h d) -> p h d", h=HPC))

            kmax = (g + 1) * NTCG
            kdiag = g * NTCG
            for p in range(NPAIR):
                Y = [psum.tile([P_, QB], F32, tag=f"Y{e}",
                               name=f"Y{g}_{p}_{e}")[:, :GW]
                     for e in range(2)]
                if not SINK_ADD:
                    for e in range(2):
                        h = 2 * p + e
                        nc.tensor.matmul(
                            Y[e][:], esr[0:1, h * P_:(h + 1) * P_],
                            ones[0:1, :GW], start=True, stop=False)
                for kc in range(kmax):
                    v = kc - kdiag
                    off = P_ * v if (kc >= kdiag and TRIM_S) else 0
                    S = psum.tile([P_, 2 * GW], F32, tag="S", bufs=2,
                                  name=f"S{g}_{p}_{kc}")
                    Pt = work.tile([P_, 2 * GW], BF16, tag="P", bufs=3,
                                   name=f"Pt{g}_{p}_{kc}")
                    for e in range(2):
                        rows = slice(D * e, D * e + D)
                        nc.tensor.matmul(
                            S[:, e * GW + off:(e + 1) * GW],
                            QKT[rows, 2 + p, kc * P_:(kc + 1) * P_],
                            QKT[rows, p, tg0 + off:tg0 + GW],
                            start=True, stop=True)
                    if off == 0:
                        nc.scalar.activation(
                            Pt[:], S[:], mybir.ActivationFunctionType.Exp,
                            scale=float(scale))
                    else:
                        for e in range(2):
                            nc.scalar.activation(
                                Pt[:, e * GW + off:(e + 1) * GW],
                                S[:, e * GW + off:(e + 1) * GW],
                                mybir.ActivationFunctionType.Exp,
                                scale=float(scale))
                    if kc >= kdiag:
                        meng = nc.gpsimd if GPSIMD_MASK else nc.vector
                        if TRIM_S:
                            for e in range(2):
                                meng.tensor_tensor(
                                    Pt[:, e * GW + off:e * GW + off + P_],
                                    Pt[:, e * GW + off:e * GW + off + P_],
                                    maskv[:], mybir.AluOpType.mult)
                                if off and not TRIM_AV:
                                    meng.memset(Pt[:, e * GW:e * GW + off], 0)
                        else:
                            w = P_ * (v + 1)
                            for e in range(2):
                                meng.tensor_tensor(
                                    Pt[:, e * GW:e * GW + w],
                                    Pt[:, e * GW:e * GW + w],
                                    mask4[:, v, :w], mybir.AluOpType.mult)
                    for e in range(2):
                        h = 2 * p + e
                        avoff = off if TRIM_AV else 0
                        nc.tensor.matmul(
                            Y[e][:, avoff:], VO[:, kc, h, :],
                            Pt[:, e * GW + avoff:(e + 1) * GW],
                            start=(kc == 0 and SINK_ADD),
                            stop=(kc == kmax - 1))
                for e in range(2):
                    h = 2 * p + e
                    # cross-base mult needs one PSUM input (walrus rejects
                    # SBUF x SBUF with differing base partitions)
                    scrA = work.tile([P_, GW], F32, tag="scrA",
                                     name=f"scrA{g}_{p}_{e}")
                    scrB = work.tile([P_, GW], F32, tag="scrB",
                                     name=f"scrB{g}_{p}_{e}")
                    # recip_approx_fast only works at partition base 0, so
                    # pull the denominators down to base 0 first
                    if SINK_ADD:
                        nc.vector.tensor_scalar_add(
                            scrA[0:D, :], Y[e][D:P_, :], es[0:D, h:h + 1])
                    else:
                        nc.vector.tensor_copy(scrA[0:D, :], Y[e][D:P_, :])
                    if FAST_RECIP:
                        nc.vector.reciprocal_approx_fast(
                            scrB[0:D, :], scrA[0:D, :])
                    else:
                        nc.vector.reciprocal(scrB[0:D, :], scrA[0:D, :])
                    nc.vector.tensor_tensor(
                        YT[D * e:D * e + D, p, tg0:tg0 + GW], Y[e][0:D, :],
                        scrB[0:D, :], mybir.AluOpType.mult)

            for tcl in range(g * NTCG, (g + 1) * NTCG):
                ob = work.tile([P_, C], BF16, tag="ob", name=f"ob{tcl}")
                for nh in range(C // QB):
                    po = psum.tile([P_, QB], F32, tag="qk", bufs=2,
                                   name=f"po{tcl}_{nh}")
                    for cch in range(2):
                        nc.tensor.matmul(
                            po[:],
                            YT[:, cch, tcl * P_:(tcl + 1) * P_],
                            wproj[:, cch, nh * QB:(nh + 1) * QB],
                            start=(cch == 0), stop=(cch == 1))
                    if nh % 2 == 0:
                        nc.scalar.copy(ob[:, nh * QB:(nh + 1) * QB], po[:])
                    else:
                        nc.vector.tensor_copy(ob[:, nh * QB:(nh + 1) * QB], po[:])
                nc.sync.dma_start(out_d.ap()[tcl * P_:(tcl + 1) * P_, :], ob[:])

    nc.compile()
    return nc


def _make_core_inputs(x, w_qkv, w_proj, sink_logit, core):
    b, g = core // 4, core % 4
    h0 = g * HPC
    HD = H * D

    xt = _bf16(np.asarray(x[b], dtype=np.float32).T)
    wq = w_qkv[:, h0 * D:(h0 + HPC) * D]
    wk = w_qkv[:, HD + h0 * D: HD + (h0 + HPC) * D]
    wvv = w_qkv[:, 2 * HD + h0 * D: 2 * HD + (h0 + HPC) * D]
    wqk = _bf16(np.concatenate([wq, wk], axis=1))
    wv = _bf16(wvv)
    wproj = _bf16(w_proj[h0 * D:(h0 + HPC) * D, :])

    es = np.zeros((P_, HPC), np.float32)
    for hh in range(HPC):
        es[:, hh] = np.exp(
            np.asarray(sink_logit[h0 + hh], dtype=np.float64)).astype(np.float32)

    esr = np.zeros((1, HPC * P_), np.float32)
    for hh in range(HPC):
        esr[0, hh * P_ + D:(hh + 1) * P_] = es[0, hh]

    mask = np.zeros((P_, P_), np.float32)
    for k in range(P_):
        mask[k, k:] = 1.0

    masks4 = np.zeros((4, P_, QB), np.float32)
    for v in range(4):
        for k in range(P_):
            masks4[v, k, P_ * v + k:] = 1.0

    return {
        "xt": xt, "wqk": wqk, "wv": wv, "wproj": wproj, "essc": es,
        "esrows": esr.astype(ml_dtypes.bfloat16),
        "ones512": np.ones((1, QB), ml_dtypes.bfloat16),
        "onesc": np.ones((P_, D), ml_dtypes.bfloat16),
        "mask": mask.astype(ml_dtypes.bfloat16),
        "masks4": masks4.astype(ml_dtypes.bfloat16),
    }


_CACHE = {}


def _get_runner():
    """Build (once) the bass program and the jitted SPMD callable."""
    if "fn" in _CACHE:
        return _CACHE["fn"], _CACHE["meta"]

    import jax
    from jax.experimental.shard_map import shard_map
    from jax.sharding import Mesh, NamedSharding, PartitionSpec

    import concourse.mybir as mybir
    from concourse.bass2jax import (_bass_exec_p, install_neuronx_cc_hook,
                                    partition_id_tensor)

    nc = _build_bass()
    install_neuronx_cc_hook()
    pid_name = nc.partition_id_tensor.name if nc.partition_id_tensor else None

    in_names, out_names, out_avals, zero_outs = [], [], [], []
    for alloc in nc.m.functions[0].allocations:
        if not isinstance(alloc, mybir.MemoryLocationSet):
            continue
        name = alloc.memorylocations[0].name
        if alloc.kind == "ExternalInput":
            if name != pid_name:
                in_names.append(name)
        elif alloc.kind == "ExternalOutput":
            out_names.append(name)
            shape = tuple(alloc.tensor_shape)
            dtype = mybir.dt.np(alloc.dtype)
            out_avals.append(jax.core.ShapedArray(shape, dtype))
            zero_outs.append(np.zeros(shape, dtype))
    n_params, n_outs = len(in_names), len(out_avals)
    all_names = in_names + out_names
    if pid_name is not None:
        all_names = all_names + [pid_name]

    def _body(*args):
        operands = list(args)
        if pid_name is not None:
            operands.append(partition_id_tensor())
        outs = _bass_exec_p.bind(
            *operands,
            out_avals=tuple(out_avals),
            in_names=tuple(all_names),
            out_names=tuple(out_names),
            lowering_input_output_aliases=(),
            sim_require_finite=True,
            sim_require_nnan=True,
            nc=nc,
        )
        return tuple(outs)

    devices = jax.devices()[:N_CORES]
    mesh = Mesh(np.asarray(devices), ("core",))
    spec = PartitionSpec("core")
    sharding = NamedSharding(mesh, spec)
    fn = jax.jit(
        shard_map(_body, mesh=mesh, in_specs=(spec,) * (n_params + n_outs),
                  out_specs=(spec,) * n_outs, check_rep=False),
        keep_unused=True)

    zeros_dev = [jax.device_put(
        np.zeros((N_CORES * z.shape[0], *z.shape[1:]), z.dtype), sharding)
        for z in zero_outs]

    meta = dict(in_names=in_names, out_names=out_names, out_avals=out_avals,
                sharding=sharding, zeros_dev=zeros_dev, jax=jax)
    _CACHE["fn"] = fn
    _CACHE["meta"] = meta
    return fn, meta


def kernel(x, w_qkv, w_proj, sink_logit):
    x = np.asarray(x, dtype=np.float32)
    w_qkv = np.asarray(w_qkv, dtype=np.float32)
    w_proj = np.asarray(w_proj, dtype=np.float32)
    sink_logit = np.asarray(sink_logit, dtype=np.float32)

    fn, meta = _get_runner()
    jax = meta["jax"]

    in_maps = [_make_core_inputs(x, w_qkv, w_proj, sink_logit, core)
               for core in range(N_CORES)]
    concat_in = [
        jax.device_put(
            np.concatenate([in_maps[c][nm] for c in range(N_CORES)], axis=0),
            meta["sharding"])
        for nm in meta["in_names"]]

    out_arrs = fn(*concat_in, *meta["zeros_dev"])
    jax.block_until_ready(out_arrs)

    i_out = meta["out_names"].index("out")
    per_core = np.asarray(out_arrs[i_out]).reshape(N_CORES, T, C)

    out = np.zeros((B, T, C), np.float64)
    for core in range(N_CORES):
        out[core // 4] += per_core[core].astype(np.float64)
    return out.astype(np.float32)


# revision 16
# speedup vs baseline: 1.0265x; 1.0265x over previous
"""Causal self-attention with sink logit on 8 Trainium2 NeuronCores.

nn_CausalSelfAttention: B=2, T=2048, C=1024, H=16, D=64.
    qkv = x @ w_qkv; per-head causal attention with a per-head sink logit in
    the softmax denominator; out = y @ w_proj.

Sharding: 8 cores = 2 batches x 4 head-groups (data-parallel over B,
tensor-parallel over heads). Each core computes its batch's qkv projection
restricted to its 4 heads, flash-style causal attention (S^T layout,
denominator via an appended ones-block in the V matmul), and the partial
output projection against its w_proj row-slice. All matmul operands are
bf16 (1 cycle/row on the PE vs 2 for fp32r; fp32 PSUM accumulation), the
diagonal blocks compute only the un-masked column suffix, the per-head sink
is added to the denominator on the vector engine right before the
reciprocal (no seed matmuls), and 1/denom uses the fast approximate DVE
reciprocal. The host sums the 4 per-head-group bf16 partials per batch in
float64 (the "all-reduce after c_proj", done host-side since the full
output is assembled host-side anyway).

kernel(**inputs) takes the FULL unsharded inputs and returns the FULL output.
"""
from contextlib import ExitStack

import numpy as np
import ml_dtypes

F32 = None
BF16 = None

P_ = 128          # partitions
QB = 512          # psum bank width (fp32)
D = 64            # head dim
HPC = 4           # heads per core
NPAIR = 2
B, T, C, H = 2, 2048, 1024, 16
N_CORES = 8


def _bf16(x):
    return np.ascontiguousarray(np.asarray(x, dtype=np.float32)).astype(
        ml_dtypes.bfloat16)


# feature flags (bisection)
TRIM_S = True     # compute only un-masked suffix of diagonal S blocks
TRIM_AV = False    # narrow AV matmuls on diagonal blocks
SINK_ADD = True    # sink via tensor_scalar_add before recip (else seed matmul)
FAST_RECIP = True  # reciprocal_approx_fast instead of reciprocal
GPSIMD_MASK = False  # causal mask mult on gpsimd (else vector)


def _build_bass():
    import concourse.mybir as mybir
    import concourse.tile as tile
    from concourse import bacc

    global F32, BF16
    F32 = mybir.dt.float32
    BF16 = mybir.dt.bfloat16

    CCH = C // P_             # C chunks
    GW = min(QB, T // 2)      # q/t group width
    NG = T // GW              # groups
    NTCG = GW // P_           # t-chunks per group
    scale = 1.0 / np.sqrt(D)

    nc = bacc.Bacc("TRN2", target_bir_lowering=False, debug=False,
                   num_devices=N_CORES)

    xt_d = nc.dram_tensor("xt", [C, T], BF16, kind="ExternalInput")
    wqk_d = nc.dram_tensor("wqk", [C, 2 * HPC * D], BF16, kind="ExternalInput")
    wv_d = nc.dram_tensor("wv", [C, HPC * D], BF16, kind="ExternalInput")
    wproj_d = nc.dram_tensor("wproj", [HPC * D, C], BF16, kind="ExternalInput")
    es_d = nc.dram_tensor("essc", [P_, HPC], F32, kind="ExternalInput")
    esr_d = nc.dram_tensor("esrows", [1, HPC * P_], BF16, kind="ExternalInput")
    ones_d = nc.dram_tensor("ones512", [1, QB], BF16, kind="ExternalInput")
    onesc_d = nc.dram_tensor("onesc", [P_, D], BF16, kind="ExternalInput")
    mask_d = nc.dram_tensor("mask", [P_, P_], BF16, kind="ExternalInput")
    masks4_d = nc.dram_tensor("masks4", [4, P_, QB], BF16, kind="ExternalInput")
    out_d = nc.dram_tensor("out", [T, C], BF16, kind="ExternalOutput")

    with tile.TileContext(nc) as tc, ExitStack() as ctx:
        pool = ctx.enter_context(tc.tile_pool(name="pool", bufs=1))
        xt_pool = ctx.enter_context(tc.tile_pool(name="xt", bufs=2 * CCH + 2))
        work = ctx.enter_context(tc.tile_pool(name="work", bufs=2))
        psum = ctx.enter_context(tc.tile_pool(name="ps", bufs=1, space="PSUM"))

        es = pool.tile([P_, HPC], F32, tag="es")
        esr = pool.tile([1, HPC * P_], BF16, tag="esr")
        ones = pool.tile([1, QB], BF16, tag="ones")
        onesc = pool.tile([P_, D], BF16, tag="onesc")
        maskv = pool.tile([P_, P_], BF16, tag="maskv")
        mask4 = pool.tile([P_, 4, QB], BF16, tag="mask4")
        nc.sync.dma_start(es[:], es_d.ap())
        nc.sync.dma_start(esr[:], esr_d.ap())
        nc.sync.dma_start(ones[:], ones_d.ap())
        nc.sync.dma_start(onesc[:], onesc_d.ap())
        nc.sync.dma_start(maskv[:], mask_d.ap())
        nc.sync.dma_start(mask4[:], masks4_d.ap().rearrange("v p q -> p v q"))

        wqk = pool.tile([P_, CCH, 2 * HPC * D], BF16, tag="wqk")
        wv = pool.tile([P_, CCH, HPC * D], BF16, tag="wv")
        wproj = pool.tile([P_, 2, C], BF16, tag="wproj")
        for c in range(CCH):
            nc.sync.dma_start(wqk[:, c, :], wqk_d.ap()[c * P_:(c + 1) * P_, :])
            nc.sync.dma_start(wv[:, c, :], wv_d.ap()[c * P_:(c + 1) * P_, :])
        nc.sync.dma_start(wproj[:], wproj_d.ap().rearrange("(co ci) m -> ci co m", ci=P_))

        QKT = pool.tile([P_, 2 * NPAIR, T], BF16, tag="qkt")
        VO = pool.tile([P_, T // P_, HPC, P_], BF16, tag="vo")
        YT = pool.tile([P_, NPAIR, T], BF16, tag="yt")

        nc.vector.tensor_copy(
            VO[:, :, :, D:P_],
            onesc[:, None, None, :].to_broadcast([P_, T // P_, HPC, D]))
        for g in range(NG):
            tg0 = g * GW
            xg = [xt_pool.tile([P_, GW], BF16, tag="xt", name=f"x{g}_{c}")
                  for c in range(CCH)]
            for c in range(CCH):
                nc.scalar.dma_start(xg[c][:], xt_d.ap()[c * P_:(c + 1) * P_,
                                                        tg0:tg0 + GW])
            for m in range(2 * NPAIR):
                ps = psum.tile([P_, GW], F32, tag="qk", bufs=2, name=f"qk{g}_{m}")
                for c in range(CCH):
                    nc.tensor.matmul(
                        ps[:], wqk[:, c, m * P_:(m + 1) * P_], xg[c][:],
                        start=(c == 0), stop=(c == CCH - 1))
                nc.vector.tensor_copy(QKT[:, m, tg0:tg0 + GW], ps[:])
            for tcl in range(NTCG):
                tc_g = g * NTCG + tcl
                ps = psum.tile([P_, HPC * D], F32, tag="qk", bufs=2,
                               name=f"vps{g}_{tcl}")
                for c in range(CCH):
                    nc.tensor.matmul(
                        ps[:], xg[c][:, tcl * P_:(tcl + 1) * P_], wv[:, c, :],
                        start=(c == 0), stop=(c == CCH - 1))
                nc.vector.tensor_copy(
                    VO[:, tc_g, :, 0:D],
                    ps[:].rearrange("p (h d) -> p h d", h=HPC))

            kmax = (g + 1) * NTCG
            kdiag = g * NTCG
            for p in range(NPAIR):
                Y = [psum.tile([P_, QB], F32, tag=f"Y{e}",
                               name=f"Y{g}_{p}_{e}")[:, :GW]
                     for e in range(2)]
                if not SINK_ADD:
                    for e in range(2):
                        h = 2 * p + e
                        nc.tensor.matmul(
                            Y[e][:], esr[0:1, h * P_:(h + 1) * P_],
                            ones[0:1, :GW], start=True, stop=False)
                for kc in range(kmax):
                    v = kc - kdiag
                    off = P_ * v if (kc >= kdiag and TRIM_S) else 0
                    S = psum.tile([P_, 2 * GW], F32, tag="S", bufs=2,
                                  name=f"S{g}_{p}_{kc}")
                    Pt = work.tile([P_, 2 * GW], BF16, tag="P", bufs=3,
                                   name=f"Pt{g}_{p}_{kc}")
                    for e in range(2):
                        rows = slice(D * e, D * e + D)
                        nc.tensor.matmul(
                            S[:, e * GW + off:(e + 1) * GW],
                            QKT[rows, 2 + p, kc * P_:(kc + 1) * P_],
                            QKT[rows, p, tg0 + off:tg0 + GW],
                            start=True, stop=True)
                    if off == 0:
                        nc.scalar.activation(
                            Pt[:], S[:], mybir.ActivationFunctionType.Exp,
                            scale=float(scale))
                    else:
                        for e in range(2):
                            nc.scalar.activation(
                                Pt[:, e * GW + off:(e + 1) * GW],
                                S[:, e * GW + off:(e + 1) * GW],
                                mybir.ActivationFunctionType.Exp,
                                scale=float(scale))
                    if kc >= kdiag:
                        meng = nc.gpsimd if GPSIMD_MASK else nc.vector
                        if TRIM_S:
                            for e in range(2):
                                meng.tensor_tensor(
                                    Pt[:, e * GW + off:e * GW + off + P_],
                                    Pt[:, e * GW + off:e * GW + off + P_],
                                    maskv[:], mybir.AluOpType.mult)
                                if off and not TRIM_AV:
                                    meng.memset(Pt[:, e * GW:e * GW + off], 0)
                        else:
                            w = P_ * (v + 1)
                            for e in range(2):
                                meng.tensor_tensor(
                                    Pt[:, e * GW:e * GW + w],
                                    Pt[:, e * GW:e * GW + w],
                                    mask4[:, v, :w], mybir.AluOpType.mult)
                    for e in range(2):
                        h = 2 * p + e
                        avoff = off if TRIM_AV else 0
                        nc.tensor.matmul(
                            Y[e][:, avoff:], VO[:, kc, h, :],
                            Pt[:, e * GW + avoff:(e + 1) * GW],
                            start=(kc == 0 and SINK_ADD),
                            stop=(kc == kmax - 1))
                for e in range(2):
                    h = 2 * p + e
                    # cross-base mult needs one PSUM input (walrus rejects
                    # SBUF x SBUF with differing base partitions)
                    scrA = work.tile([P_, GW], F32, tag="scrA",
                                     name=f"scrA{g}_{p}_{e}")
                    scrB = work.tile([P_, GW], F32, tag="scrB",
                                     name=f"scrB{g}_{p}_{e}")
                    # recip_approx_fast only works at partition base 0, so
                    # pull the denominators down to base 0 first
                    if SINK_ADD:
                        nc.vector.tensor_scalar_add(
                            scrA[0:D, :], Y[e][D:P_, :], es[0:D, h:h + 1])
                    else:
                        nc.vector.tensor_copy(scrA[0:D, :], Y[e][D:P_, :])
                    if FAST_RECIP:
                        nc.vector.reciprocal_approx_fast(
                            scrB[0:D, :], scrA[0:D, :])
                    else:
                        nc.vector.reciprocal(scrB[0:D, :], scrA[0:D, :])
                    nc.vector.tensor_tensor(
                        YT[D * e:D * e + D, p, tg0:tg0 + GW], Y[e][0:D, :],
                        scrB[0:D, :], mybir.AluOpType.mult)

            for tcl in range(g * NTCG, (g + 1) * NTCG):
                ob = work.tile([P_, C], BF16, tag="ob", name=f"ob{tcl}")
                for nh in range(C // QB):
                    po = psum.tile([P_, QB], F32, tag="qk", bufs=2,
                                   name=f"po{tcl}_{nh}")
                    for cch in range(2):
                        nc.tensor.matmul(
                            po[:],
                            YT[:, cch, tcl * P_:(tcl + 1) * P_],
                            wproj[:, cch, nh * QB:(nh + 1) * QB],
                            start=(cch == 0), stop=(cch == 1))
                    if nh % 2 == 0:
                        nc.scalar.copy(ob[:, nh * QB:(nh + 1) * QB], po[:])
                    else:
                        nc.vector.tensor_copy(ob[:, nh * QB:(nh + 1) * QB], po[:])
                nc.sync.dma_start(out_d.ap()[tcl * P_:(tcl + 1) * P_, :], ob[:])

    nc.compile()
    return nc


def _make_core_inputs(x, w_qkv, w_proj, sink_logit, core):
    b, g = core // 4, core % 4
    h0 = g * HPC
    HD = H * D

    xt = _bf16(np.asarray(x[b], dtype=np.float32).T)
    wq = w_qkv[:, h0 * D:(h0 + HPC) * D]
    wk = w_qkv[:, HD + h0 * D: HD + (h0 + HPC) * D]
    wvv = w_qkv[:, 2 * HD + h0 * D: 2 * HD + (h0 + HPC) * D]
    wqk = _bf16(np.concatenate([wq, wk], axis=1))
    wv = _bf16(wvv)
    wproj = _bf16(w_proj[h0 * D:(h0 + HPC) * D, :])

    es = np.zeros((P_, HPC), np.float32)
    for hh in range(HPC):
        es[:, hh] = np.exp(
            np.asarray(sink_logit[h0 + hh], dtype=np.float64)).astype(np.float32)

    esr = np.zeros((1, HPC * P_), np.float32)
    for hh in range(HPC):
        esr[0, hh * P_ + D:(hh + 1) * P_] = es[0, hh]

    mask = np.zeros((P_, P_), np.float32)
    for k in range(P_):
        mask[k, k:] = 1.0

    masks4 = np.zeros((4, P_, QB), np.float32)
    for v in range(4):
        for k in range(P_):
            masks4[v, k, P_ * v + k:] = 1.0

    return {
        "xt": xt, "wqk": wqk, "wv": wv, "wproj": wproj, "essc": es,
        "esrows": esr.astype(ml_dtypes.bfloat16),
        "ones512": np.ones((1, QB), ml_dtypes.bfloat16),
        "onesc": np.ones((P_, D), ml_dtypes.bfloat16),
        "mask": mask.astype(ml_dtypes.bfloat16),
        "masks4": masks4.astype(ml_dtypes.bfloat16),
    }


_CACHE = {}


def _get_runner():
    """Build (once) the bass program and the jitted SPMD callable."""
    if "fn" in _CACHE:
        return _CACHE["fn"], _CACHE["meta"]

    import jax
    from jax.experimental.shard_map import shard_map
    from jax.sharding import Mesh, NamedSharding, PartitionSpec

    import concourse.mybir as mybir
    from concourse.bass2jax import (_bass_exec_p, install_neuronx_cc_hook,
                                    partition_id_tensor)

    nc = _build_bass()
    install_neuronx_cc_hook()
    pid_name = nc.partition_id_tensor.name if nc.partition_id_tensor else None

    in_names, out_names, out_avals, zero_outs = [], [], [], []
    for alloc in nc.m.functions[0].allocations:
        if not isinstance(alloc, mybir.MemoryLocationSet):
            continue
        name = alloc.memorylocations[0].name
        if alloc.kind == "ExternalInput":
            if name != pid_name:
                in_names.append(name)
        elif alloc.kind == "ExternalOutput":
            out_names.append(name)
            shape = tuple(alloc.tensor_shape)
            dtype = mybir.dt.np(alloc.dtype)
            out_avals.append(jax.core.ShapedArray(shape, dtype))
            zero_outs.append(np.zeros(shape, dtype))
    n_params, n_outs = len(in_names), len(out_avals)
    all_names = in_names + out_names
    if pid_name is not None:
        all_names = all_names + [pid_name]

    def _body(*args):
        operands = list(args)
        if pid_name is not None:
            operands.append(partition_id_tensor())
        outs = _bass_exec_p.bind(
            *operands,
            out_avals=tuple(out_avals),
            in_names=tuple(all_names),
            out_names=tuple(out_names),
            lowering_input_output_aliases=(),
            sim_require_finite=True,
            sim_require_nnan=True,
            nc=nc,
        )
        return tuple(outs)

    devices = jax.devices()[:N_CORES]
    mesh = Mesh(np.asarray(devices), ("core",))
    spec = PartitionSpec("core")
    sharding = NamedSharding(mesh, spec)
    fn = jax.jit(
        shard_map(_body, mesh=mesh, in_specs=(spec,) * (n_params + n_outs),
                  out_specs=(spec,) * n_outs, check_rep=False),
        keep_unused=True)

    zeros_dev = [jax.device_put(
        np.zeros((N_CORES * z.shape[0], *z.shape[1:]), z.dtype), sharding)
        for z in zero_outs]

    meta = dict(in_names=in_names, out_names=out_names, out_avals=out_avals,
                sharding=sharding, zeros_dev=zeros_dev, jax=jax)
    _CACHE["fn"] = fn
    _CACHE["meta"] = meta
    return fn, meta


def kernel(x, w_qkv, w_proj, sink_logit):
    x = np.asarray(x, dtype=np.float32)
    w_qkv = np.asarray(w_qkv, dtype=np.float32)
    w_proj = np.asarray(w_proj, dtype=np.float32)
    sink_logit = np.asarray(sink_logit, dtype=np.float32)

    fn, meta = _get_runner()
    jax = meta["jax"]

    in_maps = [_make_core_inputs(x, w_qkv, w_proj, sink_logit, core)
               for core in range(N_CORES)]
    concat_in = [
        jax.device_put(
            np.concatenate([in_maps[c][nm] for c in range(N_CORES)], axis=0),
            meta["sharding"])
        for nm in meta["in_names"]]

    out_arrs = fn(*concat_in, *meta["zeros_dev"])
    jax.block_until_ready(out_arrs)

    i_out = meta["out_names"].index("out")
    per_core = np.asarray(out_arrs[i_out]).reshape(N_CORES, T, C)

    out = np.zeros((B, T, C), np.float64)
    for core in range(N_CORES):
        out[core // 4] += per_core[core].astype(np.float64)
    return out.astype(np.float32)


# revision 17
# speedup vs baseline: 1.2064x; 1.1753x over previous
"""Causal self-attention with sink logit on 8 Trainium2 NeuronCores.

nn_CausalSelfAttention: B=2, T=2048, C=1024, H=16, D=64.
    qkv = x @ w_qkv; per-head causal attention with a per-head sink logit in
    the softmax denominator; out = y @ w_proj.

Sharding: 8 cores = 2 batches x 4 head-groups (data-parallel over B,
tensor-parallel over heads). Each core computes its batch's qkv projection
restricted to its 4 heads, flash-style causal attention (S^T layout,
denominator via an appended ones-block in the V matmul), and the partial
output projection against its w_proj row-slice. All matmul operands are
bf16 (1 cycle/row on the PE vs 2 for fp32r; fp32 PSUM accumulation), the
diagonal blocks compute only the un-masked column suffix, the per-head sink
is added to the denominator on the vector engine right before the fast
approximate DVE reciprocal, and the whole thing is software-pipelined: each
attention chunk's AV matmul issues one iteration behind its exp, with the
next group's qkv and the previous group's output-projection matmuls
interleaved between chunks so the PE never waits on the activation engine.
The host sums the 4 per-head-group bf16 partials per batch in float64 (the
"all-reduce after c_proj", done host-side since the full output is
assembled host-side anyway).

kernel(**inputs) takes the FULL unsharded inputs and returns the FULL output.
"""
from contextlib import ExitStack

import numpy as np
import ml_dtypes

F32 = None
BF16 = None

P_ = 128          # partitions
QB = 512          # psum bank width (fp32)
D = 64            # head dim
HPC = 4           # heads per core
NPAIR = 2
B, T, C, H = 2, 2048, 1024, 16
N_CORES = 8


def _bf16(x):
    return np.ascontiguousarray(np.asarray(x, dtype=np.float32)).astype(
        ml_dtypes.bfloat16)


def _build_bass():
    import concourse.mybir as mybir
    import concourse.tile as tile
    from concourse import bacc

    global F32, BF16
    F32 = mybir.dt.float32
    BF16 = mybir.dt.bfloat16

    CCH = C // P_             # C chunks
    GW = min(QB, T // 2)      # q/t group width
    NG = T // GW              # groups
    NTCG = GW // P_           # t-chunks per group
    scale = 1.0 / np.sqrt(D)

    nc = bacc.Bacc("TRN2", target_bir_lowering=False, debug=False,
                   num_devices=N_CORES)

    xt_d = nc.dram_tensor("xt", [C, T], BF16, kind="ExternalInput")
    wqk_d = nc.dram_tensor("wqk", [C, 2 * HPC * D], BF16, kind="ExternalInput")
    wv_d = nc.dram_tensor("wv", [C, HPC * D], BF16, kind="ExternalInput")
    wproj_d = nc.dram_tensor("wproj", [HPC * D, C], BF16, kind="ExternalInput")
    es_d = nc.dram_tensor("essc", [P_, HPC], F32, kind="ExternalInput")
    onesc_d = nc.dram_tensor("onesc", [P_, D], BF16, kind="ExternalInput")
    mask_d = nc.dram_tensor("mask", [P_, P_], BF16, kind="ExternalInput")
    out_d = nc.dram_tensor("out", [T, C], BF16, kind="ExternalOutput")

    with tile.TileContext(nc) as tc, ExitStack() as ctx:
        pool = ctx.enter_context(tc.tile_pool(name="pool", bufs=1))
        xt_pool = ctx.enter_context(tc.tile_pool(name="xt", bufs=2))
        work = ctx.enter_context(tc.tile_pool(name="work", bufs=2))
        psum = ctx.enter_context(tc.tile_pool(name="ps", bufs=1, space="PSUM"))

        es = pool.tile([P_, HPC], F32, tag="es")
        onesc = pool.tile([P_, D], BF16, tag="onesc")
        maskv = pool.tile([P_, P_], BF16, tag="maskv")
        wqk = pool.tile([P_, CCH, 2 * HPC * D], BF16, tag="wqk")
        wv = pool.tile([P_, CCH, HPC * D], BF16, tag="wv")
        wproj = pool.tile([P_, 2, C], BF16, tag="wproj")
        QKT = pool.tile([P_, 2 * NPAIR, T], BF16, tag="qkt")
        VO = pool.tile([P_, T // P_, HPC, P_], BF16, tag="vo")
        YT = pool.tile([P_, NPAIR, T], BF16, tag="yt")

        xg_tiles = {}

        def load_x(g):
            t = xt_pool.tile([P_, CCH, GW], BF16, tag="xt", name=f"x{g}")
            nc.scalar.dma_start(
                t[:],
                xt_d.ap().rearrange("(cc p) t -> p cc t", p=P_)[
                    :, :, g * GW:(g + 1) * GW])
            xg_tiles[g] = t

        # x for group 0 first (it gates the first matmul), then weights
        load_x(0)
        nc.sync.dma_start(wqk[:], wqk_d.ap().rearrange("(cc p) m -> p cc m", p=P_))
        nc.sync.dma_start(wv[:], wv_d.ap().rearrange("(cc p) m -> p cc m", p=P_))
        nc.sync.dma_start(es[:], es_d.ap())
        nc.sync.dma_start(onesc[:], onesc_d.ap())
        nc.sync.dma_start(maskv[:], mask_d.ap())
        nc.gpsimd.dma_start(
            wproj[:], wproj_d.ap().rearrange("(co ci) m -> ci co m", ci=P_))
        nc.vector.tensor_copy(
            VO[:, :, :, D:P_],
            onesc[:, None, None, :].to_broadcast([P_, T // P_, HPC, D]))

        def qkv_ops(g):
            # deferred per-psum-group closures: 8 matmuls + 1 psum drain each
            tg0 = g * GW
            ops = []

            def qk_op(m, g=g, tg0=tg0):
                xg = xg_tiles[g]
                ps = psum.tile([P_, GW], F32, tag="qk", bufs=2,
                               name=f"qk{g}_{m}")
                for c in range(CCH):
                    nc.tensor.matmul(
                        ps[:], wqk[:, c, m * P_:(m + 1) * P_], xg[:, c, :],
                        start=(c == 0), stop=(c == CCH - 1))
                nc.vector.tensor_copy(QKT[:, m, tg0:tg0 + GW], ps[:])

            def v_op(tcl, g=g):
                xg = xg_tiles[g]
                tc_g = g * NTCG + tcl
                ps = psum.tile([P_, HPC * D], F32, tag="qk", bufs=2,
                               name=f"vps{g}_{tcl}")
                for c in range(CCH):
                    nc.tensor.matmul(
                        ps[:], xg[:, c, tcl * P_:(tcl + 1) * P_], wv[:, c, :],
                        start=(c == 0), stop=(c == CCH - 1))
                nc.vector.tensor_copy(
                    VO[:, tc_g, :, 0:D],
                    ps[:].rearrange("p (h d) -> p h d", h=HPC))

            for m in range(2 * NPAIR):
                ops.append(lambda m=m: qk_op(m))
            for tcl in range(NTCG):
                ops.append(lambda tcl=tcl: v_op(tcl))
            return ops

        def proj_ops(g):
            ops = []

            def tcl_op(tcl):
                ob = work.tile([P_, C], BF16, tag="ob", name=f"ob{tcl}")
                for nh in range(C // QB):
                    po = psum.tile([P_, QB], F32, tag="qk", bufs=2,
                                   name=f"po{tcl}_{nh}")
                    for cch in range(2):
                        nc.tensor.matmul(
                            po[:],
                            YT[:, cch, tcl * P_:(tcl + 1) * P_],
                            wproj[:, cch, nh * QB:(nh + 1) * QB],
                            start=(cch == 0), stop=(cch == 1))
                    nc.vector.tensor_copy(ob[:, nh * QB:(nh + 1) * QB], po[:])
                nc.gpsimd.dma_start(out_d.ap()[tcl * P_:(tcl + 1) * P_, :],
                                    ob[:])

            for tcl in range(g * NTCG, (g + 1) * NTCG):
                ops.append(lambda tcl=tcl: tcl_op(tcl))
            return ops

        for op in qkv_ops(0):
            op()

        for g in range(NG):
            tg0 = g * GW
            if g + 1 < NG:
                load_x(g + 1)
            fill = []
            if g + 1 < NG:
                fill.extend(qkv_ops(g + 1))
            if g >= 1:
                fill.extend(proj_ops(g - 1))
            fill.reverse()   # pop() serves in original order

            kmax = (g + 1) * NTCG
            kdiag = g * NTCG
            for p in range(NPAIR):
                Y = [psum.tile([P_, QB], F32, tag=f"Y{e}",
                               name=f"Y{g}_{p}_{e}")[:, :GW]
                     for e in range(2)]
                pend = None

                def av(kc, Pt, p=p, Y=Y, kmax=kmax):
                    for e in range(2):
                        h = 2 * p + e
                        nc.tensor.matmul(
                            Y[e][:], VO[:, kc, h, :],
                            Pt[:, e * GW:(e + 1) * GW],
                            start=(kc == 0), stop=(kc == kmax - 1))

                for kc in range(kmax):
                    v = kc - kdiag
                    off = P_ * v if kc >= kdiag else 0
                    S = psum.tile([P_, 2 * GW], F32, tag="S", bufs=2,
                                  name=f"S{g}_{p}_{kc}")
                    Pt = work.tile([P_, 2 * GW], BF16, tag="P", bufs=3,
                                   name=f"Pt{g}_{p}_{kc}")
                    for e in range(2):
                        rows = slice(D * e, D * e + D)
                        nc.tensor.matmul(
                            S[:, e * GW + off:(e + 1) * GW],
                            QKT[rows, 2 + p, kc * P_:(kc + 1) * P_],
                            QKT[rows, p, tg0 + off:tg0 + GW],
                            start=True, stop=True)
                    if off == 0:
                        nc.scalar.activation(
                            Pt[:], S[:], mybir.ActivationFunctionType.Exp,
                            scale=float(scale))
                    else:
                        for e in range(2):
                            nc.scalar.activation(
                                Pt[:, e * GW + off:(e + 1) * GW],
                                S[:, e * GW + off:(e + 1) * GW],
                                mybir.ActivationFunctionType.Exp,
                                scale=float(scale))
                    if kc >= kdiag:
                        for e in range(2):
                            nc.vector.tensor_tensor(
                                Pt[:, e * GW + off:e * GW + off + P_],
                                Pt[:, e * GW + off:e * GW + off + P_],
                                maskv[:], mybir.AluOpType.mult)
                            if off:
                                nc.vector.memset(Pt[:, e * GW:e * GW + off], 0)
                    # software pipeline: AV for the previous chunk, then one
                    # deferred qkv/proj psum-group to keep the PE fed while
                    # this chunk's exp runs on the activation engine
                    if pend is not None:
                        av(*pend)
                    if fill and (g > 0 or kc >= 2):
                        fill.pop()()
                    pend = (kc, Pt)
                av(*pend)

                for e in range(2):
                    h = 2 * p + e
                    scrA = work.tile([P_, GW], F32, tag="scrA",
                                     name=f"scrA{g}_{p}_{e}")
                    scrB = work.tile([P_, GW], F32, tag="scrB",
                                     name=f"scrB{g}_{p}_{e}")
                    # denom += exp(sink); fast approx reciprocal (base-0
                    # partitions only); cross-base mult needs one PSUM input
                    nc.vector.tensor_scalar_add(
                        scrA[0:D, :], Y[e][D:P_, :], es[0:D, h:h + 1])
                    nc.vector.reciprocal_approx_fast(
                        scrB[0:D, :], scrA[0:D, :])
                    nc.vector.tensor_tensor(
                        YT[D * e:D * e + D, p, tg0:tg0 + GW], Y[e][0:D, :],
                        scrB[0:D, :], mybir.AluOpType.mult)
            while fill:
                fill.pop()()
        for op in proj_ops(NG - 1):
            op()

    nc.compile()
    return nc


def _make_core_inputs(x, w_qkv, w_proj, sink_logit, core):
    b, g = core // 4, core % 4
    h0 = g * HPC
    HD = H * D

    xt = _bf16(np.asarray(x[b], dtype=np.float32).T)
    wq = w_qkv[:, h0 * D:(h0 + HPC) * D]
    wk = w_qkv[:, HD + h0 * D: HD + (h0 + HPC) * D]
    wvv = w_qkv[:, 2 * HD + h0 * D: 2 * HD + (h0 + HPC) * D]
    wqk = _bf16(np.concatenate([wq, wk], axis=1))
    wv = _bf16(wvv)
    wproj = _bf16(w_proj[h0 * D:(h0 + HPC) * D, :])

    es = np.zeros((P_, HPC), np.float32)
    for hh in range(HPC):
        es[:, hh] = np.exp(
            np.asarray(sink_logit[h0 + hh], dtype=np.float64)).astype(np.float32)

    mask = np.zeros((P_, P_), np.float32)
    for k in range(P_):
        mask[k, k:] = 1.0

    return {
        "xt": xt, "wqk": wqk, "wv": wv, "wproj": wproj, "essc": es,
        "onesc": np.ones((P_, D), ml_dtypes.bfloat16),
        "mask": mask.astype(ml_dtypes.bfloat16),
    }


_CACHE = {}


def _get_runner():
    """Build (once) the bass program and the jitted SPMD callable."""
    if "fn" in _CACHE:
        return _CACHE["fn"], _CACHE["meta"]

    import jax
    from jax.experimental.shard_map import shard_map
    from jax.sharding import Mesh, NamedSharding, PartitionSpec

    import concourse.mybir as mybir
    from concourse.bass2jax import (_bass_exec_p, install_neuronx_cc_hook,
                                    partition_id_tensor)

    nc = _build_bass()
    install_neuronx_cc_hook()
    pid_name = nc.partition_id_tensor.name if nc.partition_id_tensor else None

    in_names, out_names, out_avals, zero_outs = [], [], [], []
    for alloc in nc.m.functions[0].allocations:
        if not isinstance(alloc, mybir.MemoryLocationSet):
            continue
        name = alloc.memorylocations[0].name
        if alloc.kind == "ExternalInput":
            if name != pid_name:
                in_names.append(name)
        elif alloc.kind == "ExternalOutput":
            out_names.append(name)
            shape = tuple(alloc.tensor_shape)
            dtype = mybir.dt.np(alloc.dtype)
            out_avals.append(jax.core.ShapedArray(shape, dtype))
            zero_outs.append(np.zeros(shape, dtype))
    n_params, n_outs = len(in_names), len(out_avals)
    all_names = in_names + out_names
    if pid_name is not None:
        all_names = all_names + [pid_name]

    def _body(*args):
        operands = list(args)
        if pid_name is not None:
            operands.append(partition_id_tensor())
        outs = _bass_exec_p.bind(
            *operands,
            out_avals=tuple(out_avals),
            in_names=tuple(all_names),
            out_names=tuple(out_names),
            lowering_input_output_aliases=(),
            sim_require_finite=True,
            sim_require_nnan=True,
            nc=nc,
        )
        return tuple(outs)

    devices = jax.devices()[:N_CORES]
    mesh = Mesh(np.asarray(devices), ("core",))
    spec = PartitionSpec("core")
    sharding = NamedSharding(mesh, spec)
    fn = jax.jit(
        shard_map(_body, mesh=mesh, in_specs=(spec,) * (n_params + n_outs),
                  out_specs=(spec,) * n_outs, check_rep=False),
        keep_unused=True)

    zeros_dev = [jax.device_put(
        np.zeros((N_CORES * z.shape[0], *z.shape[1:]), z.dtype), sharding)
        for z in zero_outs]

    meta = dict(in_names=in_names, out_names=out_names, out_avals=out_avals,
                sharding=sharding, zeros_dev=zeros_dev, jax=jax)
    _CACHE["fn"] = fn
    _CACHE["meta"] = meta
    return fn, meta


def kernel(x, w_qkv, w_proj, sink_logit):
    x = np.asarray(x, dtype=np.float32)
    w_qkv = np.asarray(w_qkv, dtype=np.float32)
    w_proj = np.asarray(w_proj, dtype=np.float32)
    sink_logit = np.asarray(sink_logit, dtype=np.float32)

    fn, meta = _get_runner()
    jax = meta["jax"]

    in_maps = [_make_core_inputs(x, w_qkv, w_proj, sink_logit, core)
               for core in range(N_CORES)]
    concat_in = [
        jax.device_put(
            np.concatenate([in_maps[c][nm] for c in range(N_CORES)], axis=0),
            meta["sharding"])
        for nm in meta["in_names"]]

    out_arrs = fn(*concat_in, *meta["zeros_dev"])
    jax.block_until_ready(out_arrs)

    i_out = meta["out_names"].index("out")
    per_core = np.asarray(out_arrs[i_out]).reshape(N_CORES, T, C)

    out = np.zeros((B, T, C), np.float64)
    for core in range(N_CORES):
        out[core // 4] += per_core[core].astype(np.float64)
    return out.astype(np.float32)


# revision 19
# speedup vs baseline: 1.2205x; 1.0117x over previous
"""Causal self-attention with sink logit on 8 Trainium2 NeuronCores.

nn_CausalSelfAttention: B=2, T=2048, C=1024, H=16, D=64.
    qkv = x @ w_qkv; per-head causal attention with a per-head sink logit in
    the softmax denominator; out = y @ w_proj.

Sharding: 8 cores = 2 batches x 4 head-groups (data-parallel over B,
tensor-parallel over heads). Each core computes its batch's qkv projection
restricted to its 4 heads, flash-style causal attention (S^T layout,
denominator via an appended ones-block in the V matmul), and the partial
output projection against its w_proj row-slice. All matmul operands are
bf16 (1 cycle/row on the PE vs 2 for fp32r; fp32 PSUM accumulation), the
diagonal blocks compute only the un-masked column suffix, the per-head sink
is added to the denominator on the vector engine right before the fast
approximate DVE reciprocal, and the whole thing is software-pipelined: each
attention chunk's AV matmul issues one iteration behind its exp, with the
next group's qkv and the previous group's output-projection matmuls
interleaved between chunks so the PE never waits on the activation engine.
The host sums the 4 per-head-group bf16 partials per batch in float64 (the
"all-reduce after c_proj", done host-side since the full output is
assembled host-side anyway).

kernel(**inputs) takes the FULL unsharded inputs and returns the FULL output.
"""
from contextlib import ExitStack

import numpy as np
import ml_dtypes

F32 = None
BF16 = None

P_ = 128          # partitions
QB = 512          # psum bank width (fp32)
D = 64            # head dim
HPC = 4           # heads per core
NPAIR = 2
B, T, C, H = 2, 2048, 1024, 16
N_CORES = 8


def _bf16(x):
    return np.ascontiguousarray(np.asarray(x, dtype=np.float32)).astype(
        ml_dtypes.bfloat16)


def _build_bass():
    import concourse.mybir as mybir
    import concourse.tile as tile
    from concourse import bacc

    global F32, BF16
    F32 = mybir.dt.float32
    BF16 = mybir.dt.bfloat16

    CCH = C // P_             # C chunks
    GW = min(QB, T // 2)      # q/t group width
    NG = T // GW              # groups
    NTCG = GW // P_           # t-chunks per group
    scale = 1.0 / np.sqrt(D)

    nc = bacc.Bacc("TRN2", target_bir_lowering=False, debug=False,
                   num_devices=N_CORES)

    xt_d = nc.dram_tensor("xt", [C, T], BF16, kind="ExternalInput")
    wqk_d = nc.dram_tensor("wqk", [C, 2 * HPC * D], BF16, kind="ExternalInput")
    wv_d = nc.dram_tensor("wv", [C, HPC * D], BF16, kind="ExternalInput")
    wproj_d = nc.dram_tensor("wproj", [HPC * D, C], BF16, kind="ExternalInput")
    es_d = nc.dram_tensor("essc", [P_, HPC], F32, kind="ExternalInput")
    onesc_d = nc.dram_tensor("onesc", [P_, D], BF16, kind="ExternalInput")
    mask_d = nc.dram_tensor("mask", [P_, P_], BF16, kind="ExternalInput")
    out_d = nc.dram_tensor("out", [T, C], BF16, kind="ExternalOutput")

    with tile.TileContext(nc) as tc, ExitStack() as ctx:
        pool = ctx.enter_context(tc.tile_pool(name="pool", bufs=1))
        xt_pool = ctx.enter_context(tc.tile_pool(name="xt", bufs=2))
        work = ctx.enter_context(tc.tile_pool(name="work", bufs=2))
        psum = ctx.enter_context(tc.tile_pool(name="ps", bufs=1, space="PSUM"))

        es = pool.tile([P_, HPC], F32, tag="es")
        onesc = pool.tile([P_, D], BF16, tag="onesc")
        maskv = pool.tile([P_, P_], BF16, tag="maskv")
        wqk = pool.tile([P_, CCH, 2 * HPC * D], BF16, tag="wqk")
        wv = pool.tile([P_, CCH, HPC * D], BF16, tag="wv")
        wproj = pool.tile([P_, 2, C], BF16, tag="wproj")
        QKT = pool.tile([P_, 2 * NPAIR, T], BF16, tag="qkt")
        VO = pool.tile([P_, T // P_, HPC, P_], BF16, tag="vo")
        YT = pool.tile([P_, NPAIR, T], BF16, tag="yt")

        xg_tiles = {}
        xt_re = xt_d.ap().rearrange("(cc p) t -> p cc t", p=P_)

        def load_x(g, split=False):
            t = xt_pool.tile([P_, CCH, GW], BF16, tag="xt", name=f"x{g}")
            if split:
                nc.scalar.dma_start(
                    t[:, 0:CCH // 2, :],
                    xt_re[:, 0:CCH // 2, g * GW:(g + 1) * GW])
                nc.scalar.dma_start(
                    t[:, CCH // 2:CCH, :],
                    xt_re[:, CCH // 2:CCH, g * GW:(g + 1) * GW])
            else:
                nc.scalar.dma_start(t[:], xt_re[:, :, g * GW:(g + 1) * GW])
            xg_tiles[g] = t

        # startup: halve the first x/wqk transfers so the first qk matmul
        # chain only waits on ~512KB per queue, not the full 2MB
        load_x(0, split=True)
        HW2 = HPC * D
        nc.sync.dma_start(
            wqk[:, :, 0:HW2],
            wqk_d.ap()[:, 0:HW2].rearrange("(cc p) m -> p cc m", p=P_))
        nc.sync.dma_start(
            wqk[:, :, HW2:2 * HW2],
            wqk_d.ap()[:, HW2:2 * HW2].rearrange("(cc p) m -> p cc m", p=P_))
        nc.sync.dma_start(wv[:], wv_d.ap().rearrange("(cc p) m -> p cc m", p=P_))
        nc.sync.dma_start(es[:], es_d.ap())
        nc.sync.dma_start(onesc[:], onesc_d.ap())
        nc.sync.dma_start(maskv[:], mask_d.ap())
        nc.gpsimd.dma_start(
            wproj[:], wproj_d.ap().rearrange("(co ci) m -> ci co m", ci=P_))
        nc.vector.tensor_copy(
            VO[:, :, :, D:P_],
            onesc[:, None, None, :].to_broadcast([P_, T // P_, HPC, D]))

        def qkv_ops(g):
            # deferred per-psum-group closures: 8 matmuls + 1 psum drain each
            tg0 = g * GW
            ops = []

            def qk_op(m, g=g, tg0=tg0):
                xg = xg_tiles[g]
                ps = psum.tile([P_, GW], F32, tag="qk", bufs=2,
                               name=f"qk{g}_{m}")
                for c in range(CCH):
                    nc.tensor.matmul(
                        ps[:], wqk[:, c, m * P_:(m + 1) * P_], xg[:, c, :],
                        start=(c == 0), stop=(c == CCH - 1))
                nc.vector.tensor_copy(QKT[:, m, tg0:tg0 + GW], ps[:])

            def v_op(tcl, g=g):
                xg = xg_tiles[g]
                tc_g = g * NTCG + tcl
                ps = psum.tile([P_, HPC * D], F32, tag="qk", bufs=2,
                               name=f"vps{g}_{tcl}")
                for c in range(CCH):
                    nc.tensor.matmul(
                        ps[:], xg[:, c, tcl * P_:(tcl + 1) * P_], wv[:, c, :],
                        start=(c == 0), stop=(c == CCH - 1))
                nc.vector.tensor_copy(
                    VO[:, tc_g, :, 0:D],
                    ps[:].rearrange("p (h d) -> p h d", h=HPC))

            for m in range(2 * NPAIR):
                ops.append(lambda m=m: qk_op(m))
            for tcl in range(NTCG):
                ops.append(lambda tcl=tcl: v_op(tcl))
            return ops

        def proj_ops(g):
            ops = []

            def tcl_op(tcl):
                ob = work.tile([P_, C], BF16, tag="ob", name=f"ob{tcl}")
                for nh in range(C // QB):
                    po = psum.tile([P_, QB], F32, tag="qk", bufs=2,
                                   name=f"po{tcl}_{nh}")
                    for cch in range(2):
                        nc.tensor.matmul(
                            po[:],
                            YT[:, cch, tcl * P_:(tcl + 1) * P_],
                            wproj[:, cch, nh * QB:(nh + 1) * QB],
                            start=(cch == 0), stop=(cch == 1))
                    nc.vector.tensor_copy(ob[:, nh * QB:(nh + 1) * QB], po[:])
                nc.gpsimd.dma_start(out_d.ap()[tcl * P_:(tcl + 1) * P_, :],
                                    ob[:])

            for tcl in range(g * NTCG, (g + 1) * NTCG):
                ops.append(lambda tcl=tcl: tcl_op(tcl))
            return ops

        for op in qkv_ops(0):
            op()

        for g in range(NG):
            tg0 = g * GW
            if g + 1 < NG:
                load_x(g + 1)
            fill = []
            if g + 1 < NG:
                fill.extend(qkv_ops(g + 1))
            if g >= 1:
                fill.extend(proj_ops(g - 1))
            fill.reverse()   # pop() serves in original order

            kmax = (g + 1) * NTCG
            kdiag = g * NTCG
            for p in range(NPAIR):
                Y = [psum.tile([P_, QB], F32, tag=f"Y{e}",
                               name=f"Y{g}_{p}_{e}")[:, :GW]
                     for e in range(2)]
                pend = []

                def av(kc, Pt, p=p, Y=Y, kmax=kmax):
                    for e in range(2):
                        h = 2 * p + e
                        nc.tensor.matmul(
                            Y[e][:], VO[:, kc, h, :],
                            Pt[:, e * GW:(e + 1) * GW],
                            start=(kc == 0), stop=(kc == kmax - 1))

                for kc in range(kmax):
                    v = kc - kdiag
                    off = P_ * v if kc >= kdiag else 0
                    S = psum.tile([P_, 2 * GW], F32, tag="S", bufs=2,
                                  name=f"S{g}_{p}_{kc}")
                    Pt = work.tile([P_, 2 * GW], BF16, tag="P", bufs=4,
                                   name=f"Pt{g}_{p}_{kc}")
                    for e in range(2):
                        rows = slice(D * e, D * e + D)
                        nc.tensor.matmul(
                            S[:, e * GW + off:(e + 1) * GW],
                            QKT[rows, 2 + p, kc * P_:(kc + 1) * P_],
                            QKT[rows, p, tg0 + off:tg0 + GW],
                            start=True, stop=True)
                    if off == 0:
                        nc.scalar.activation(
                            Pt[:], S[:], mybir.ActivationFunctionType.Exp,
                            scale=float(scale))
                    else:
                        for e in range(2):
                            nc.scalar.activation(
                                Pt[:, e * GW + off:(e + 1) * GW],
                                S[:, e * GW + off:(e + 1) * GW],
                                mybir.ActivationFunctionType.Exp,
                                scale=float(scale))
                    if kc >= kdiag:
                        for e in range(2):
                            nc.gpsimd.tensor_tensor(
                                Pt[:, e * GW + off:e * GW + off + P_],
                                Pt[:, e * GW + off:e * GW + off + P_],
                                maskv[:], mybir.AluOpType.mult)
                            if off:
                                nc.gpsimd.memset(Pt[:, e * GW:e * GW + off], 0)
                    # software pipeline: AV runs two chunks behind its exp,
                    # with one deferred qkv/proj psum-group interleaved so
                    # the PE never waits on the activation engine
                    if len(pend) >= 2:
                        av(*pend.pop(0))
                    if fill and (g > 0 or kc >= 1):
                        fill.pop()()
                    pend.append((kc, Pt))
                while pend:
                    av(*pend.pop(0))

                for e in range(2):
                    h = 2 * p + e
                    scrA = work.tile([P_, GW], F32, tag="scrA",
                                     name=f"scrA{g}_{p}_{e}")
                    scrB = work.tile([P_, GW], F32, tag="scrB",
                                     name=f"scrB{g}_{p}_{e}")
                    # denom += exp(sink); fast approx reciprocal (base-0
                    # partitions only); cross-base mult needs one PSUM input
                    nc.vector.tensor_scalar_add(
                        scrA[0:D, :], Y[e][D:P_, :], es[0:D, h:h + 1])
                    nc.vector.reciprocal_approx_fast(
                        scrB[0:D, :], scrA[0:D, :])
                    nc.vector.tensor_tensor(
                        YT[D * e:D * e + D, p, tg0:tg0 + GW], Y[e][0:D, :],
                        scrB[0:D, :], mybir.AluOpType.mult)
            while fill:
                fill.pop()()
        for op in proj_ops(NG - 1):
            op()

    nc.compile()
    return nc


def _make_core_inputs(x, w_qkv, w_proj, sink_logit, core):
    b, g = core // 4, core % 4
    h0 = g * HPC
    HD = H * D

    xt = _bf16(np.asarray(x[b], dtype=np.float32).T)
    wq = w_qkv[:, h0 * D:(h0 + HPC) * D]
    wk = w_qkv[:, HD + h0 * D: HD + (h0 + HPC) * D]
    wvv = w_qkv[:, 2 * HD + h0 * D: 2 * HD + (h0 + HPC) * D]
    wqk = _bf16(np.concatenate([wq, wk], axis=1))
    wv = _bf16(wvv)
    wproj = _bf16(w_proj[h0 * D:(h0 + HPC) * D, :])

    es = np.zeros((P_, HPC), np.float32)
    for hh in range(HPC):
        es[:, hh] = np.exp(
            np.asarray(sink_logit[h0 + hh], dtype=np.float64)).astype(np.float32)

    mask = np.zeros((P_, P_), np.float32)
    for k in range(P_):
        mask[k, k:] = 1.0

    return {
        "xt": xt, "wqk": wqk, "wv": wv, "wproj": wproj, "essc": es,
        "onesc": np.ones((P_, D), ml_dtypes.bfloat16),
        "mask": mask.astype(ml_dtypes.bfloat16),
    }


_CACHE = {}


def _get_runner():
    """Build (once) the bass program and the jitted SPMD callable."""
    if "fn" in _CACHE:
        return _CACHE["fn"], _CACHE["meta"]

    import jax
    from jax.experimental.shard_map import shard_map
    from jax.sharding import Mesh, NamedSharding, PartitionSpec

    import concourse.mybir as mybir
    from concourse.bass2jax import (_bass_exec_p, install_neuronx_cc_hook,
                                    partition_id_tensor)

    nc = _build_bass()
    install_neuronx_cc_hook()
    pid_name = nc.partition_id_tensor.name if nc.partition_id_tensor else None

    in_names, out_names, out_avals, zero_outs = [], [], [], []
    for alloc in nc.m.functions[0].allocations:
        if not isinstance(alloc, mybir.MemoryLocationSet):
            continue
        name = alloc.memorylocations[0].name
        if alloc.kind == "ExternalInput":
            if name != pid_name:
                in_names.append(name)
        elif alloc.kind == "ExternalOutput":
            out_names.append(name)
            shape = tuple(alloc.tensor_shape)
            dtype = mybir.dt.np(alloc.dtype)
            out_avals.append(jax.core.ShapedArray(shape, dtype))
            zero_outs.append(np.zeros(shape, dtype))
    n_params, n_outs = len(in_names), len(out_avals)
    all_names = in_names + out_names
    if pid_name is not None:
        all_names = all_names + [pid_name]

    def _body(*args):
        operands = list(args)
        if pid_name is not None:
            operands.append(partition_id_tensor())
        outs = _bass_exec_p.bind(
            *operands,
            out_avals=tuple(out_avals),
            in_names=tuple(all_names),
            out_names=tuple(out_names),
            lowering_input_output_aliases=(),
            sim_require_finite=True,
            sim_require_nnan=True,
            nc=nc,
        )
        return tuple(outs)

    devices = jax.devices()[:N_CORES]
    mesh = Mesh(np.asarray(devices), ("core",))
    spec = PartitionSpec("core")
    sharding = NamedSharding(mesh, spec)
    fn = jax.jit(
        shard_map(_body, mesh=mesh, in_specs=(spec,) * (n_params + n_outs),
                  out_specs=(spec,) * n_outs, check_rep=False),
        keep_unused=True)

    zeros_dev = [jax.device_put(
        np.zeros((N_CORES * z.shape[0], *z.shape[1:]), z.dtype), sharding)
        for z in zero_outs]

    meta = dict(in_names=in_names, out_names=out_names, out_avals=out_avals,
                sharding=sharding, zeros_dev=zeros_dev, jax=jax)
    _CACHE["fn"] = fn
    _CACHE["meta"] = meta
    return fn, meta


def kernel(x, w_qkv, w_proj, sink_logit):
    x = np.asarray(x, dtype=np.float32)
    w_qkv = np.asarray(w_qkv, dtype=np.float32)
    w_proj = np.asarray(w_proj, dtype=np.float32)
    sink_logit = np.asarray(sink_logit, dtype=np.float32)

    fn, meta = _get_runner()
    jax = meta["jax"]

    in_maps = [_make_core_inputs(x, w_qkv, w_proj, sink_logit, core)
               for core in range(N_CORES)]
    concat_in = [
        jax.device_put(
            np.concatenate([in_maps[c][nm] for c in range(N_CORES)], axis=0),
            meta["sharding"])
        for nm in meta["in_names"]]

    out_arrs = fn(*concat_in, *meta["zeros_dev"])
    jax.block_until_ready(out_arrs)

    i_out = meta["out_names"].index("out")
    per_core = np.asarray(out_arrs[i_out]).reshape(N_CORES, T, C)

    out = np.zeros((B, T, C), np.float64)
    for core in range(N_CORES):
        out[core // 4] += per_core[core].astype(np.float64)
    return out.astype(np.float32)


# revision 25
# speedup vs baseline: 1.2788x; 1.0478x over previous
"""Causal self-attention with sink logit on 8 Trainium2 NeuronCores.

nn_CausalSelfAttention: B=2, T=2048, C=1024, H=16, D=64.
    qkv = x @ w_qkv; per-head causal attention with a per-head sink logit in
    the softmax denominator; out = y @ w_proj.

Sharding: 8 cores = 2 batches x 4 head-groups (data-parallel over B,
tensor-parallel over heads). Each core computes its batch's qkv projection
restricted to its 4 heads, flash-style causal attention (S^T layout,
denominator via an appended ones-block in the V matmul), and the partial
output projection against its w_proj row-slice. All matmul operands are
bf16 (1 cycle/row on the PE vs 2 for fp32r; fp32 PSUM accumulation), the
diagonal blocks compute only the un-masked column suffix, the per-head sink
is added to the denominator on the vector engine right before the fast
approximate DVE reciprocal, and the whole thing is software-pipelined: each
attention chunk's AV matmul issues one iteration behind its exp, with the
next group's qkv and the previous group's output-projection matmuls
interleaved between chunks so the PE never waits on the activation engine.
The host sums the 4 per-head-group bf16 partials per batch in float64 (the
"all-reduce after c_proj", done host-side since the full output is
assembled host-side anyway).

kernel(**inputs) takes the FULL unsharded inputs and returns the FULL output.
"""
from contextlib import ExitStack

import numpy as np
import ml_dtypes

F32 = None
BF16 = None

P_ = 128          # partitions
QB = 512          # psum bank width (fp32)
D = 64            # head dim
HPC = 4           # heads per core
NPAIR = 2
B, T, C, H = 2, 2048, 1024, 16
N_CORES = 8


def _bf16(x):
    return np.ascontiguousarray(np.asarray(x, dtype=np.float32)).astype(
        ml_dtypes.bfloat16)


def _build_bass():
    import concourse.mybir as mybir
    import concourse.tile as tile
    from concourse import bacc

    global F32, BF16
    F32 = mybir.dt.float32
    BF16 = mybir.dt.bfloat16

    CCH = C // P_             # C chunks
    GW = min(QB, T // 2)      # q/t group width
    NG = T // GW              # groups
    NTCG = GW // P_           # t-chunks per group
    scale = 1.0 / np.sqrt(D)

    nc = bacc.Bacc("TRN2", target_bir_lowering=False, debug=False,
                   num_devices=N_CORES)

    xt_d = nc.dram_tensor("xt", [C, T], BF16, kind="ExternalInput")
    wqk_d = nc.dram_tensor("wqk", [C, 2 * HPC * D], BF16, kind="ExternalInput")
    wv_d = nc.dram_tensor("wv", [C, HPC * D], BF16, kind="ExternalInput")
    wproj_d = nc.dram_tensor("wproj", [HPC * D, C], BF16, kind="ExternalInput")
    es_d = nc.dram_tensor("essc", [P_, HPC], F32, kind="ExternalInput")
    onesc_d = nc.dram_tensor("onesc", [P_, D], BF16, kind="ExternalInput")
    mask_d = nc.dram_tensor("mask", [P_, P_], BF16, kind="ExternalInput")
    out_d = nc.dram_tensor("out", [T, C], BF16, kind="ExternalOutput")

    with tile.TileContext(nc) as tc, ExitStack() as ctx:
        pool = ctx.enter_context(tc.tile_pool(name="pool", bufs=1))
        xt_pool = ctx.enter_context(tc.tile_pool(name="xt", bufs=3))
        work = ctx.enter_context(tc.tile_pool(name="work", bufs=2))
        psum = ctx.enter_context(tc.tile_pool(name="ps", bufs=1, space="PSUM"))

        es = pool.tile([P_, HPC], F32, tag="es")
        onesc = pool.tile([P_, D], BF16, tag="onesc")
        maskv = pool.tile([P_, P_], BF16, tag="maskv")
        wqk = pool.tile([P_, CCH, 2 * HPC * D], BF16, tag="wqk")
        wv = pool.tile([P_, CCH, HPC * D], BF16, tag="wv")
        wproj = pool.tile([P_, 2, C], BF16, tag="wproj")
        QKT = pool.tile([P_, 2 * NPAIR, T], BF16, tag="qkt")
        VO = pool.tile([P_, T // P_, HPC, P_], BF16, tag="vo")
        YT = pool.tile([P_, NPAIR, T], BF16, tag="yt")

        xg_tiles = {}
        xt_re = xt_d.ap().rearrange("(cc p) t -> p cc t", p=P_)

        def load_x(g, split=False):
            t = xt_pool.tile([P_, CCH, GW], BF16, tag="xt", name=f"x{g}")
            if split:
                nc.scalar.dma_start(
                    t[:, 0:CCH // 2, :],
                    xt_re[:, 0:CCH // 2, g * GW:(g + 1) * GW])
                nc.scalar.dma_start(
                    t[:, CCH // 2:CCH, :],
                    xt_re[:, CCH // 2:CCH, g * GW:(g + 1) * GW])
            else:
                nc.scalar.dma_start(t[:], xt_re[:, :, g * GW:(g + 1) * GW])
            xg_tiles[g] = t

        # startup: halve the first x/wqk transfers so the first qk matmul
        # chain only waits on ~512KB per queue, not the full 2MB; small/late
        # tensors (es has a pathologically slow 16B-per-partition descriptor)
        # go on the gpsimd queue where nothing early waits
        load_x(0, split=True)
        HW2 = HPC * D
        nc.sync.dma_start(
            wqk[:, :, 0:HW2],
            wqk_d.ap()[:, 0:HW2].rearrange("(cc p) m -> p cc m", p=P_))
        nc.sync.dma_start(
            wqk[:, :, HW2:2 * HW2],
            wqk_d.ap()[:, HW2:2 * HW2].rearrange("(cc p) m -> p cc m", p=P_))
        nc.sync.dma_start(wv[:], wv_d.ap().rearrange("(cc p) m -> p cc m", p=P_))
        nc.gpsimd.dma_start(onesc[:], onesc_d.ap())
        nc.gpsimd.dma_start(maskv[:], mask_d.ap())
        nc.gpsimd.dma_start(es[:], es_d.ap())
        nc.gpsimd.dma_start(
            wproj[:], wproj_d.ap().rearrange("(co ci) m -> ci co m", ci=P_))
        nc.vector.tensor_copy(
            VO[:, :, :, D:P_],
            onesc[:, None, None, :].to_broadcast([P_, T // P_, HPC, D]))
        load_x(1)

        def qkv_ops(g):
            # deferred per-psum-group closures: 8 matmuls + 1 psum drain each
            tg0 = g * GW
            ops = []

            def qk_op(m, g=g, tg0=tg0):
                xg = xg_tiles[g]
                ps = psum.tile([P_, GW], F32, tag="qk", bufs=2,
                               name=f"qk{g}_{m}")
                for c in range(CCH):
                    nc.tensor.matmul(
                        ps[:], wqk[:, c, m * P_:(m + 1) * P_], xg[:, c, :],
                        start=(c == 0), stop=(c == CCH - 1))
                nc.vector.tensor_copy(QKT[:, m, tg0:tg0 + GW], ps[:])

            def v_op(tcl, g=g):
                xg = xg_tiles[g]
                tc_g = g * NTCG + tcl
                ps = psum.tile([P_, HPC * D], F32, tag="qk", bufs=2,
                               name=f"vps{g}_{tcl}")
                for c in range(CCH):
                    nc.tensor.matmul(
                        ps[:], xg[:, c, tcl * P_:(tcl + 1) * P_], wv[:, c, :],
                        start=(c == 0), stop=(c == CCH - 1))
                nc.vector.tensor_copy(
                    VO[:, tc_g, :, 0:D],
                    ps[:].rearrange("p (h d) -> p h d", h=HPC))

            for m in range(2 * NPAIR):
                ops.append(lambda m=m: qk_op(m))
            for tcl in range(NTCG):
                ops.append(lambda tcl=tcl: v_op(tcl))
            return ops

        def proj_ops(g, last=False):
            ops = []

            def tcl_op(tcl):
                ob = work.tile([P_, C], BF16, tag="ob", name=f"ob{tcl}")
                for nh in range(C // QB):
                    po = psum.tile([P_, QB], F32, tag="qk", bufs=2,
                                   name=f"po{tcl}_{nh}")
                    for cch in range(2):
                        nc.tensor.matmul(
                            po[:],
                            YT[:, cch, tcl * P_:(tcl + 1) * P_],
                            wproj[:, cch, nh * QB:(nh + 1) * QB],
                            start=(cch == 0), stop=(cch == 1))
                    # in the interleaved (non-last) case the scalar engine is
                    # busy with exp; at the tail it's idle, so split copies
                    if last and nh % 2 == 0:
                        nc.scalar.copy(ob[:, nh * QB:(nh + 1) * QB], po[:])
                    else:
                        nc.vector.tensor_copy(ob[:, nh * QB:(nh + 1) * QB],
                                              po[:])
                dq = (nc.sync if (last and tcl % 2 == 0) else nc.gpsimd)
                dq.dma_start(out_d.ap()[tcl * P_:(tcl + 1) * P_, :], ob[:])

            for tcl in range(g * NTCG, (g + 1) * NTCG):
                ops.append(lambda tcl=tcl: tcl_op(tcl))
            return ops

        for op in qkv_ops(0):
            op()

        for g in range(NG):
            tg0 = g * GW
            if g + 2 < NG:
                load_x(g + 2)
            fill = []
            if g + 1 < NG:
                fill.extend(qkv_ops(g + 1))
            if g >= 1:
                fill.extend(proj_ops(g - 1))
            fill.reverse()   # pop() serves in original order

            kmax = (g + 1) * NTCG
            kdiag = g * NTCG
            for p in range(NPAIR):
                Y = [psum.tile([P_, QB], F32, tag=f"Y{e}",
                               name=f"Y{g}_{p}_{e}")[:, :GW]
                     for e in range(2)]
                pend = []

                def av(kc, Pt, p=p, Y=Y, kmax=kmax):
                    for e in range(2):
                        h = 2 * p + e
                        nc.tensor.matmul(
                            Y[e][:], VO[:, kc, h, :],
                            Pt[:, e * GW:(e + 1) * GW],
                            start=(kc == 0), stop=(kc == kmax - 1))

                for kc in range(kmax):
                    v = kc - kdiag
                    off = P_ * v if kc >= kdiag else 0
                    S = psum.tile([P_, 2 * GW], F32, tag="S", bufs=2,
                                  name=f"S{g}_{p}_{kc}")
                    Pt = work.tile([P_, 2 * GW], BF16, tag="P", bufs=4,
                                   name=f"Pt{g}_{p}_{kc}")
                    for e in range(2):
                        rows = slice(D * e, D * e + D)
                        nc.tensor.matmul(
                            S[:, e * GW + off:(e + 1) * GW],
                            QKT[rows, 2 + p, kc * P_:(kc + 1) * P_],
                            QKT[rows, p, tg0 + off:tg0 + GW],
                            start=True, stop=True)
                    if off == 0:
                        nc.scalar.activation(
                            Pt[:], S[:], mybir.ActivationFunctionType.Exp,
                            scale=float(scale))
                    else:
                        for e in range(2):
                            nc.scalar.activation(
                                Pt[:, e * GW + off:(e + 1) * GW],
                                S[:, e * GW + off:(e + 1) * GW],
                                mybir.ActivationFunctionType.Exp,
                                scale=float(scale))
                    if kc >= kdiag:
                        for e in range(2):
                            nc.vector.tensor_tensor(
                                Pt[:, e * GW + off:e * GW + off + P_],
                                Pt[:, e * GW + off:e * GW + off + P_],
                                maskv[:], mybir.AluOpType.mult)
                            if off:
                                nc.vector.memset(Pt[:, e * GW:e * GW + off], 0)
                    # software pipeline: AV runs two chunks behind its exp,
                    # with one deferred qkv/proj psum-group interleaved so
                    # the PE never waits on the activation engine
                    if len(pend) >= 2:
                        av(*pend.pop(0))
                    if fill and (g > 0 or kc >= 1):
                        fill.pop()()
                    pend.append((kc, Pt))
                while pend:
                    av(*pend.pop(0))

                for e in range(2):
                    h = 2 * p + e
                    scrA = work.tile([P_, GW], F32, tag="scrA",
                                     name=f"scrA{g}_{p}_{e}")
                    scrB = work.tile([P_, GW], F32, tag="scrB",
                                     name=f"scrB{g}_{p}_{e}")
                    # denom += exp(sink); fast approx reciprocal (base-0
                    # partitions only); cross-base mult needs one PSUM input
                    nc.vector.tensor_scalar_add(
                        scrA[0:D, :], Y[e][D:P_, :], es[0:D, h:h + 1])
                    nc.vector.reciprocal_approx_fast(
                        scrB[0:D, :], scrA[0:D, :])
                    nc.vector.tensor_tensor(
                        YT[D * e:D * e + D, p, tg0:tg0 + GW], Y[e][0:D, :],
                        scrB[0:D, :], mybir.AluOpType.mult)
            while fill:
                fill.pop()()
        for op in proj_ops(NG - 1, last=True):
            op()

    nc.compile()
    return nc


def _make_core_inputs(x, w_qkv, w_proj, sink_logit, core):
    b, g = core // 4, core % 4
    h0 = g * HPC
    HD = H * D

    xt = _bf16(np.asarray(x[b], dtype=np.float32).T)
    wq = w_qkv[:, h0 * D:(h0 + HPC) * D]
    wk = w_qkv[:, HD + h0 * D: HD + (h0 + HPC) * D]
    wvv = w_qkv[:, 2 * HD + h0 * D: 2 * HD + (h0 + HPC) * D]
    wqk = _bf16(np.concatenate([wq, wk], axis=1))
    wv = _bf16(wvv)
    wproj = _bf16(w_proj[h0 * D:(h0 + HPC) * D, :])

    es = np.zeros((P_, HPC), np.float32)
    for hh in range(HPC):
        es[:, hh] = np.exp(
            np.asarray(sink_logit[h0 + hh], dtype=np.float64)).astype(np.float32)

    mask = np.zeros((P_, P_), np.float32)
    for k in range(P_):
        mask[k, k:] = 1.0

    return {
        "xt": xt, "wqk": wqk, "wv": wv, "wproj": wproj, "essc": es,
        "onesc": np.ones((P_, D), ml_dtypes.bfloat16),
        "mask": mask.astype(ml_dtypes.bfloat16),
    }


_CACHE = {}


def _get_runner():
    """Build (once) the bass program and the jitted SPMD callable."""
    if "fn" in _CACHE:
        return _CACHE["fn"], _CACHE["meta"]

    import jax
    from jax.experimental.shard_map import shard_map
    from jax.sharding import Mesh, NamedSharding, PartitionSpec

    import concourse.mybir as mybir
    from concourse.bass2jax import (_bass_exec_p, install_neuronx_cc_hook,
                                    partition_id_tensor)

    nc = _build_bass()
    install_neuronx_cc_hook()
    pid_name = nc.partition_id_tensor.name if nc.partition_id_tensor else None

    in_names, out_names, out_avals, zero_outs = [], [], [], []
    for alloc in nc.m.functions[0].allocations:
        if not isinstance(alloc, mybir.MemoryLocationSet):
            continue
        name = alloc.memorylocations[0].name
        if alloc.kind == "ExternalInput":
            if name != pid_name:
                in_names.append(name)
        elif alloc.kind == "ExternalOutput":
            out_names.append(name)
            shape = tuple(alloc.tensor_shape)
            dtype = mybir.dt.np(alloc.dtype)
            out_avals.append(jax.core.ShapedArray(shape, dtype))
            zero_outs.append(np.zeros(shape, dtype))
    n_params, n_outs = len(in_names), len(out_avals)
    all_names = in_names + out_names
    if pid_name is not None:
        all_names = all_names + [pid_name]

    def _body(*args):
        operands = list(args)
        if pid_name is not None:
            operands.append(partition_id_tensor())
        outs = _bass_exec_p.bind(
            *operands,
            out_avals=tuple(out_avals),
            in_names=tuple(all_names),
            out_names=tuple(out_names),
            lowering_input_output_aliases=(),
            sim_require_finite=True,
            sim_require_nnan=True,
            nc=nc,
        )
        return tuple(outs)

    devices = jax.devices()[:N_CORES]
    mesh = Mesh(np.asarray(devices), ("core",))
    spec = PartitionSpec("core")
    sharding = NamedSharding(mesh, spec)
    fn = jax.jit(
        shard_map(_body, mesh=mesh, in_specs=(spec,) * (n_params + n_outs),
                  out_specs=(spec,) * n_outs, check_rep=False),
        keep_unused=True)

    zeros_dev = [jax.device_put(
        np.zeros((N_CORES * z.shape[0], *z.shape[1:]), z.dtype), sharding)
        for z in zero_outs]

    meta = dict(in_names=in_names, out_names=out_names, out_avals=out_avals,
                sharding=sharding, zeros_dev=zeros_dev, jax=jax)
    _CACHE["fn"] = fn
    _CACHE["meta"] = meta
    return fn, meta


def kernel(x, w_qkv, w_proj, sink_logit):
    x = np.asarray(x, dtype=np.float32)
    w_qkv = np.asarray(w_qkv, dtype=np.float32)
    w_proj = np.asarray(w_proj, dtype=np.float32)
    sink_logit = np.asarray(sink_logit, dtype=np.float32)

    fn, meta = _get_runner()
    jax = meta["jax"]

    in_maps = [_make_core_inputs(x, w_qkv, w_proj, sink_logit, core)
               for core in range(N_CORES)]
    concat_in = [
        jax.device_put(
            np.concatenate([in_maps[c][nm] for c in range(N_CORES)], axis=0),
            meta["sharding"])
        for nm in meta["in_names"]]

    out_arrs = fn(*concat_in, *meta["zeros_dev"])
    jax.block_until_ready(out_arrs)

    i_out = meta["out_names"].index("out")
    per_core = np.asarray(out_arrs[i_out]).reshape(N_CORES, T, C)

    out = np.zeros((B, T, C), np.float64)
    for core in range(N_CORES):
        out[core // 4] += per_core[core].astype(np.float64)
    return out.astype(np.float32)


# revision 27
# speedup vs baseline: 1.2935x; 1.0115x over previous
"""Causal self-attention with sink logit on 8 Trainium2 NeuronCores.

nn_CausalSelfAttention: B=2, T=2048, C=1024, H=16, D=64.
    qkv = x @ w_qkv; per-head causal attention with a per-head sink logit in
    the softmax denominator; out = y @ w_proj.

Sharding: 8 cores = 2 batches x 4 head-groups (data-parallel over B,
tensor-parallel over heads). Each core computes its batch's qkv projection
restricted to its 4 heads, flash-style causal attention (S^T layout,
denominator via an appended ones-block in the V matmul), and the partial
output projection against its w_proj row-slice. All matmul operands are
bf16 (1 cycle/row on the PE vs 2 for fp32r; fp32 PSUM accumulation), the
diagonal blocks compute only the un-masked column suffix, the per-head sink
is added to the denominator on the vector engine right before the fast
approximate DVE reciprocal, and the whole thing is software-pipelined: each
attention chunk's AV matmul issues one iteration behind its exp, with the
next group's qkv and the previous group's output-projection matmuls
interleaved between chunks so the PE never waits on the activation engine.
The host sums the 4 per-head-group bf16 partials per batch in float64 (the
"all-reduce after c_proj", done host-side since the full output is
assembled host-side anyway).

kernel(**inputs) takes the FULL unsharded inputs and returns the FULL output.
"""
from contextlib import ExitStack

import numpy as np
import ml_dtypes

F32 = None
BF16 = None

P_ = 128          # partitions
QB = 512          # psum bank width (fp32)
D = 64            # head dim
HPC = 4           # heads per core
NPAIR = 2
B, T, C, H = 2, 2048, 1024, 16
N_CORES = 8


def _bf16(x):
    return np.ascontiguousarray(np.asarray(x, dtype=np.float32)).astype(
        ml_dtypes.bfloat16)


def _build_bass():
    import concourse.mybir as mybir
    import concourse.tile as tile
    from concourse import bacc

    global F32, BF16
    F32 = mybir.dt.float32
    BF16 = mybir.dt.bfloat16

    CCH = C // P_             # C chunks
    GW = min(QB, T // 2)      # q/t group width
    NG = T // GW              # groups
    NTCG = GW // P_           # t-chunks per group
    scale = 1.0 / np.sqrt(D)

    nc = bacc.Bacc("TRN2", target_bir_lowering=False, debug=False,
                   num_devices=N_CORES)

    xt_d = nc.dram_tensor("xt", [C, T], BF16, kind="ExternalInput")
    wqk_d = nc.dram_tensor("wqk", [C, 2 * HPC * D], BF16, kind="ExternalInput")
    wv_d = nc.dram_tensor("wv", [C, HPC * D], BF16, kind="ExternalInput")
    wproj_d = nc.dram_tensor("wproj", [HPC * D, C], BF16, kind="ExternalInput")
    es_d = nc.dram_tensor("essc", [P_, HPC], F32, kind="ExternalInput")
    onesc_d = nc.dram_tensor("onesc", [P_, D], BF16, kind="ExternalInput")
    mask_d = nc.dram_tensor("mask", [P_, P_], BF16, kind="ExternalInput")
    out_d = nc.dram_tensor("out", [T, C], BF16, kind="ExternalOutput")

    with tile.TileContext(nc) as tc, ExitStack() as ctx:
        pool = ctx.enter_context(tc.tile_pool(name="pool", bufs=1))
        xt_pool = ctx.enter_context(tc.tile_pool(name="xt", bufs=3))
        work = ctx.enter_context(tc.tile_pool(name="work", bufs=2))
        psum = ctx.enter_context(tc.tile_pool(name="ps", bufs=1, space="PSUM"))

        es = pool.tile([P_, HPC], F32, tag="es")
        onesc = pool.tile([P_, D], BF16, tag="onesc")
        maskv = pool.tile([P_, P_], BF16, tag="maskv")
        wqk = pool.tile([P_, CCH, 2 * HPC * D], BF16, tag="wqk")
        wv = pool.tile([P_, CCH, HPC * D], BF16, tag="wv")
        wproj = pool.tile([P_, 2, C], BF16, tag="wproj")
        QKT = pool.tile([P_, 2 * NPAIR, T], BF16, tag="qkt")
        VO = pool.tile([P_, T // P_, HPC, P_], BF16, tag="vo")
        YT = pool.tile([P_, NPAIR, T], BF16, tag="yt")

        xg_tiles = {}
        xt_re = xt_d.ap().rearrange("(cc p) t -> p cc t", p=P_)

        def load_x(g, split=False):
            t = xt_pool.tile([P_, CCH, GW], BF16, tag="xt", name=f"x{g}")
            if split:
                nc.scalar.dma_start(
                    t[:, 0:CCH // 2, :],
                    xt_re[:, 0:CCH // 2, g * GW:(g + 1) * GW])
                nc.scalar.dma_start(
                    t[:, CCH // 2:CCH, :],
                    xt_re[:, CCH // 2:CCH, g * GW:(g + 1) * GW])
            else:
                nc.scalar.dma_start(t[:], xt_re[:, :, g * GW:(g + 1) * GW])
            xg_tiles[g] = t

        # startup: halve the first x/wqk transfers so the first qk matmul
        # chain only waits on ~512KB per queue, not the full 2MB; small/late
        # tensors (es has a pathologically slow 16B-per-partition descriptor)
        # go on the gpsimd queue where nothing early waits
        load_x(0, split=True)
        HW2 = HPC * D
        nc.sync.dma_start(
            wqk[:, :, 0:HW2],
            wqk_d.ap()[:, 0:HW2].rearrange("(cc p) m -> p cc m", p=P_))
        nc.sync.dma_start(
            wqk[:, :, HW2:2 * HW2],
            wqk_d.ap()[:, HW2:2 * HW2].rearrange("(cc p) m -> p cc m", p=P_))
        nc.sync.dma_start(wv[:], wv_d.ap().rearrange("(cc p) m -> p cc m", p=P_))
        nc.gpsimd.dma_start(onesc[:], onesc_d.ap())
        nc.gpsimd.dma_start(maskv[:], mask_d.ap())
        nc.gpsimd.dma_start(es[:], es_d.ap())
        nc.gpsimd.dma_start(
            wproj[:], wproj_d.ap().rearrange("(co ci) m -> ci co m", ci=P_))
        nc.vector.tensor_copy(
            VO[:, :, :, D:P_],
            onesc[:, None, None, :].to_broadcast([P_, T // P_, HPC, D]))
        load_x(1)

        def qkv_ops(g):
            # deferred per-psum-group closures: 8 matmuls + 1 psum drain each
            tg0 = g * GW
            ops = []

            def qk_op(m, g=g, tg0=tg0):
                xg = xg_tiles[g]
                ps = psum.tile([P_, GW], F32, tag="qk", bufs=2,
                               name=f"qk{g}_{m}")
                for c in range(CCH):
                    nc.tensor.matmul(
                        ps[:], wqk[:, c, m * P_:(m + 1) * P_], xg[:, c, :],
                        start=(c == 0), stop=(c == CCH - 1))
                nc.vector.tensor_copy(QKT[:, m, tg0:tg0 + GW], ps[:])

            def v_op(tcl, g=g):
                xg = xg_tiles[g]
                tc_g = g * NTCG + tcl
                ps = psum.tile([P_, HPC * D], F32, tag="qk", bufs=2,
                               name=f"vps{g}_{tcl}")
                for c in range(CCH):
                    nc.tensor.matmul(
                        ps[:], xg[:, c, tcl * P_:(tcl + 1) * P_], wv[:, c, :],
                        start=(c == 0), stop=(c == CCH - 1))
                nc.vector.tensor_copy(
                    VO[:, tc_g, :, 0:D],
                    ps[:].rearrange("p (h d) -> p h d", h=HPC))

            for m in range(2 * NPAIR):
                ops.append(lambda m=m: qk_op(m))
            for tcl in range(NTCG):
                ops.append(lambda tcl=tcl: v_op(tcl))
            return ops

        def proj_ops(g, last=False):
            ops = []

            def tcl_op(tcl):
                ob = work.tile([P_, C], BF16, tag="ob", name=f"ob{tcl}")
                for nh in range(C // QB):
                    po = psum.tile([P_, QB], F32, tag="qk", bufs=2,
                                   name=f"po{tcl}_{nh}")
                    for cch in range(2):
                        nc.tensor.matmul(
                            po[:],
                            YT[:, cch, tcl * P_:(tcl + 1) * P_],
                            wproj[:, cch, nh * QB:(nh + 1) * QB],
                            start=(cch == 0), stop=(cch == 1))
                    # in the interleaved (non-last) case the scalar engine is
                    # busy with exp; at the tail it's idle, so split copies
                    if last and nh % 2 == 0:
                        nc.scalar.copy(ob[:, nh * QB:(nh + 1) * QB], po[:])
                    else:
                        nc.vector.tensor_copy(ob[:, nh * QB:(nh + 1) * QB],
                                              po[:])
                dq = (nc.sync if (last and tcl % 2 == 0) else nc.gpsimd)
                dq.dma_start(out_d.ap()[tcl * P_:(tcl + 1) * P_, :], ob[:])

            for tcl in range(g * NTCG, (g + 1) * NTCG):
                ops.append(lambda tcl=tcl: tcl_op(tcl))
            return ops

        for op in qkv_ops(0):
            op()

        for g in range(NG):
            tg0 = g * GW
            if g + 2 < NG:
                load_x(g + 2)
            fill = []
            if g + 1 < NG:
                fill.extend(qkv_ops(g + 1))
            if g >= 1:
                fill.extend(proj_ops(g - 1))
            fill.reverse()   # pop() serves in original order

            kmax = (g + 1) * NTCG
            kdiag = g * NTCG
            # spread deferred work across the group's chunk iterations so the
            # PE stays fed through the normalize/proj boundary at group end
            n_iters = kmax * NPAIR - (1 if g == 0 else 0)
            stride = max(1, n_iters // (len(fill) + 1)) if fill else 1
            it_ctr = 0
            for p in range(NPAIR):
                Y = [psum.tile([P_, QB], F32, tag=f"Y{e}",
                               name=f"Y{g}_{p}_{e}")[:, :GW]
                     for e in range(2)]
                pend = []

                def av(kc, Pt, p=p, Y=Y, kmax=kmax):
                    for e in range(2):
                        h = 2 * p + e
                        nc.tensor.matmul(
                            Y[e][:], VO[:, kc, h, :],
                            Pt[:, e * GW:(e + 1) * GW],
                            start=(kc == 0), stop=(kc == kmax - 1))

                for kc in range(kmax):
                    v = kc - kdiag
                    off = P_ * v if kc >= kdiag else 0
                    S = psum.tile([P_, 2 * GW], F32, tag="S", bufs=2,
                                  name=f"S{g}_{p}_{kc}")
                    Pt = work.tile([P_, 2 * GW], BF16, tag="P", bufs=4,
                                   name=f"Pt{g}_{p}_{kc}")
                    for e in range(2):
                        rows = slice(D * e, D * e + D)
                        nc.tensor.matmul(
                            S[:, e * GW + off:(e + 1) * GW],
                            QKT[rows, 2 + p, kc * P_:(kc + 1) * P_],
                            QKT[rows, p, tg0 + off:tg0 + GW],
                            start=True, stop=True)
                    if off == 0:
                        nc.scalar.activation(
                            Pt[:], S[:], mybir.ActivationFunctionType.Exp,
                            scale=float(scale))
                    else:
                        for e in range(2):
                            nc.scalar.activation(
                                Pt[:, e * GW + off:(e + 1) * GW],
                                S[:, e * GW + off:(e + 1) * GW],
                                mybir.ActivationFunctionType.Exp,
                                scale=float(scale))
                    if kc >= kdiag:
                        for e in range(2):
                            nc.vector.tensor_tensor(
                                Pt[:, e * GW + off:e * GW + off + P_],
                                Pt[:, e * GW + off:e * GW + off + P_],
                                maskv[:], mybir.AluOpType.mult)
                            if off:
                                nc.vector.memset(Pt[:, e * GW:e * GW + off], 0)
                    # software pipeline: AV runs two chunks behind its exp,
                    # with one deferred qkv/proj psum-group interleaved so
                    # the PE never waits on the activation engine
                    if len(pend) >= 2:
                        av(*pend.pop(0))
                    if fill and (g > 0 or kc >= 1):
                        it_ctr += 1
                        if it_ctr % stride == 0:
                            fill.pop()()
                    pend.append((kc, Pt))
                while pend:
                    av(*pend.pop(0))

                for e in range(2):
                    h = 2 * p + e
                    scrA = work.tile([P_, GW], F32, tag="scrA",
                                     name=f"scrA{g}_{p}_{e}")
                    scrB = work.tile([P_, GW], F32, tag="scrB",
                                     name=f"scrB{g}_{p}_{e}")
                    # denom += exp(sink); fast approx reciprocal (base-0
                    # partitions only); cross-base mult needs one PSUM input
                    nc.vector.tensor_scalar_add(
                        scrA[0:D, :], Y[e][D:P_, :], es[0:D, h:h + 1])
                    nc.vector.reciprocal_approx_fast(
                        scrB[0:D, :], scrA[0:D, :])
                    nc.vector.tensor_tensor(
                        YT[D * e:D * e + D, p, tg0:tg0 + GW], Y[e][0:D, :],
                        scrB[0:D, :], mybir.AluOpType.mult)
            while fill:
                fill.pop()()
        for op in proj_ops(NG - 1, last=True):
            op()

    nc.compile()
    return nc


def _make_core_inputs(x, w_qkv, w_proj, sink_logit, core):
    b, g = core // 4, core % 4
    h0 = g * HPC
    HD = H * D

    xt = _bf16(np.asarray(x[b], dtype=np.float32).T)
    wq = w_qkv[:, h0 * D:(h0 + HPC) * D]
    wk = w_qkv[:, HD + h0 * D: HD + (h0 + HPC) * D]
    wvv = w_qkv[:, 2 * HD + h0 * D: 2 * HD + (h0 + HPC) * D]
    wqk = _bf16(np.concatenate([wq, wk], axis=1))
    wv = _bf16(wvv)
    wproj = _bf16(w_proj[h0 * D:(h0 + HPC) * D, :])

    es = np.zeros((P_, HPC), np.float32)
    for hh in range(HPC):
        es[:, hh] = np.exp(
            np.asarray(sink_logit[h0 + hh], dtype=np.float64)).astype(np.float32)

    mask = np.zeros((P_, P_), np.float32)
    for k in range(P_):
        mask[k, k:] = 1.0

    return {
        "xt": xt, "wqk": wqk, "wv": wv, "wproj": wproj, "essc": es,
        "onesc": np.ones((P_, D), ml_dtypes.bfloat16),
        "mask": mask.astype(ml_dtypes.bfloat16),
    }


_CACHE = {}


def _get_runner():
    """Build (once) the bass program and the jitted SPMD callable."""
    if "fn" in _CACHE:
        return _CACHE["fn"], _CACHE["meta"]

    import jax
    from jax.experimental.shard_map import shard_map
    from jax.sharding import Mesh, NamedSharding, PartitionSpec

    import concourse.mybir as mybir
    from concourse.bass2jax import (_bass_exec_p, install_neuronx_cc_hook,
                                    partition_id_tensor)

    nc = _build_bass()
    install_neuronx_cc_hook()
    pid_name = nc.partition_id_tensor.name if nc.partition_id_tensor else None

    in_names, out_names, out_avals, zero_outs = [], [], [], []
    for alloc in nc.m.functions[0].allocations:
        if not isinstance(alloc, mybir.MemoryLocationSet):
            continue
        name = alloc.memorylocations[0].name
        if alloc.kind == "ExternalInput":
            if name != pid_name:
                in_names.append(name)
        elif alloc.kind == "ExternalOutput":
            out_names.append(name)
            shape = tuple(alloc.tensor_shape)
            dtype = mybir.dt.np(alloc.dtype)
            out_avals.append(jax.core.ShapedArray(shape, dtype))
            zero_outs.append(np.zeros(shape, dtype))
    n_params, n_outs = len(in_names), len(out_avals)
    all_names = in_names + out_names
    if pid_name is not None:
        all_names = all_names + [pid_name]

    def _body(*args):
        operands = list(args)
        if pid_name is not None:
            operands.append(partition_id_tensor())
        outs = _bass_exec_p.bind(
            *operands,
            out_avals=tuple(out_avals),
            in_names=tuple(all_names),
            out_names=tuple(out_names),
            lowering_input_output_aliases=(),
            sim_require_finite=True,
            sim_require_nnan=True,
            nc=nc,
        )
        return tuple(outs)

    devices = jax.devices()[:N_CORES]
    mesh = Mesh(np.asarray(devices), ("core",))
    spec = PartitionSpec("core")
    sharding = NamedSharding(mesh, spec)
    fn = jax.jit(
        shard_map(_body, mesh=mesh, in_specs=(spec,) * (n_params + n_outs),
                  out_specs=(spec,) * n_outs, check_rep=False),
        keep_unused=True)

    zeros_dev = [jax.device_put(
        np.zeros((N_CORES * z.shape[0], *z.shape[1:]), z.dtype), sharding)
        for z in zero_outs]

    meta = dict(in_names=in_names, out_names=out_names, out_avals=out_avals,
                sharding=sharding, zeros_dev=zeros_dev, jax=jax)
    _CACHE["fn"] = fn
    _CACHE["meta"] = meta
    return fn, meta


def kernel(x, w_qkv, w_proj, sink_logit):
    x = np.asarray(x, dtype=np.float32)
    w_qkv = np.asarray(w_qkv, dtype=np.float32)
    w_proj = np.asarray(w_proj, dtype=np.float32)
    sink_logit = np.asarray(sink_logit, dtype=np.float32)

    fn, meta = _get_runner()
    jax = meta["jax"]

    in_maps = [_make_core_inputs(x, w_qkv, w_proj, sink_logit, core)
               for core in range(N_CORES)]
    concat_in = [
        jax.device_put(
            np.concatenate([in_maps[c][nm] for c in range(N_CORES)], axis=0),
            meta["sharding"])
        for nm in meta["in_names"]]

    out_arrs = fn(*concat_in, *meta["zeros_dev"])
    jax.block_until_ready(out_arrs)

    i_out = meta["out_names"].index("out")
    per_core = np.asarray(out_arrs[i_out]).reshape(N_CORES, T, C)

    out = np.zeros((B, T, C), np.float64)
    for core in range(N_CORES):
        out[core // 4] += per_core[core].astype(np.float64)
    return out.astype(np.float32)


# revision 28
# speedup vs baseline: 1.3045x; 1.0085x over previous
"""Causal self-attention with sink logit on 8 Trainium2 NeuronCores.

nn_CausalSelfAttention: B=2, T=2048, C=1024, H=16, D=64.
    qkv = x @ w_qkv; per-head causal attention with a per-head sink logit in
    the softmax denominator; out = y @ w_proj.

Sharding: 8 cores = 2 batches x 4 head-groups (data-parallel over B,
tensor-parallel over heads). Each core computes its batch's qkv projection
restricted to its 4 heads, flash-style causal attention (S^T layout,
denominator via an appended ones-block in the V matmul), and the partial
output projection against its w_proj row-slice. All matmul operands are
bf16 (1 cycle/row on the PE vs 2 for fp32r; fp32 PSUM accumulation), the
diagonal blocks compute only the un-masked column suffix, the per-head sink
is added to the denominator on the vector engine right before the fast
approximate DVE reciprocal, and the whole thing is software-pipelined: each
attention chunk's AV matmul issues one iteration behind its exp, with the
next group's qkv and the previous group's output-projection matmuls
interleaved between chunks so the PE never waits on the activation engine.
The host sums the 4 per-head-group bf16 partials per batch in float64 (the
"all-reduce after c_proj", done host-side since the full output is
assembled host-side anyway).

kernel(**inputs) takes the FULL unsharded inputs and returns the FULL output.
"""
from contextlib import ExitStack

import numpy as np
import ml_dtypes

F32 = None
BF16 = None

P_ = 128          # partitions
QB = 512          # psum bank width (fp32)
D = 64            # head dim
HPC = 4           # heads per core
NPAIR = 2
B, T, C, H = 2, 2048, 1024, 16
N_CORES = 8


def _bf16(x):
    return np.ascontiguousarray(np.asarray(x, dtype=np.float32)).astype(
        ml_dtypes.bfloat16)


def _build_bass():
    import concourse.mybir as mybir
    import concourse.tile as tile
    from concourse import bacc

    global F32, BF16
    F32 = mybir.dt.float32
    BF16 = mybir.dt.bfloat16

    CCH = C // P_             # C chunks
    GW = min(QB, T // 2)      # q/t group width
    NG = T // GW              # groups
    NTCG = GW // P_           # t-chunks per group
    scale = 1.0 / np.sqrt(D)

    nc = bacc.Bacc("TRN2", target_bir_lowering=False, debug=False,
                   num_devices=N_CORES)

    xt_d = nc.dram_tensor("xt", [C, T], BF16, kind="ExternalInput")
    wqk_d = nc.dram_tensor("wqk", [C, 2 * HPC * D], BF16, kind="ExternalInput")
    wv_d = nc.dram_tensor("wv", [C, HPC * D], BF16, kind="ExternalInput")
    wproj_d = nc.dram_tensor("wproj", [HPC * D, C], BF16, kind="ExternalInput")
    es_d = nc.dram_tensor("essc", [P_, HPC], F32, kind="ExternalInput")
    onesc_d = nc.dram_tensor("onesc", [P_, D], BF16, kind="ExternalInput")
    mask_d = nc.dram_tensor("mask", [P_, P_], BF16, kind="ExternalInput")
    out_d = nc.dram_tensor("out", [T, C], BF16, kind="ExternalOutput")

    with tile.TileContext(nc) as tc, ExitStack() as ctx:
        pool = ctx.enter_context(tc.tile_pool(name="pool", bufs=1))
        xt_pool = ctx.enter_context(tc.tile_pool(name="xt", bufs=3))
        work = ctx.enter_context(tc.tile_pool(name="work", bufs=2))
        psum = ctx.enter_context(tc.tile_pool(name="ps", bufs=1, space="PSUM"))

        es = pool.tile([P_, HPC], F32, tag="es")
        onesc = pool.tile([P_, D], BF16, tag="onesc")
        maskv = pool.tile([P_, P_], BF16, tag="maskv")
        wqk = pool.tile([P_, CCH, 2 * HPC * D], BF16, tag="wqk")
        wv = pool.tile([P_, CCH, HPC * D], BF16, tag="wv")
        wproj = pool.tile([P_, 2, C], BF16, tag="wproj")
        QKT = pool.tile([P_, 2 * NPAIR, T], BF16, tag="qkt")
        VO = pool.tile([P_, T // P_, HPC, P_], BF16, tag="vo")
        YT = pool.tile([P_, NPAIR, T], BF16, tag="yt")

        xg_tiles = {}
        xt_re = xt_d.ap().rearrange("(cc p) t -> p cc t", p=P_)

        def load_x(g, split=False):
            t = xt_pool.tile([P_, CCH, GW], BF16, tag="xt", name=f"x{g}")
            if split:
                nc.scalar.dma_start(
                    t[:, 0:CCH // 2, :],
                    xt_re[:, 0:CCH // 2, g * GW:(g + 1) * GW])
                nc.scalar.dma_start(
                    t[:, CCH // 2:CCH, :],
                    xt_re[:, CCH // 2:CCH, g * GW:(g + 1) * GW])
            else:
                nc.scalar.dma_start(t[:], xt_re[:, :, g * GW:(g + 1) * GW])
            xg_tiles[g] = t

        # startup: halve the first x/wqk transfers so the first qk matmul
        # chain only waits on ~512KB per queue, not the full 2MB; small/late
        # tensors (es has a pathologically slow 16B-per-partition descriptor)
        # go on the gpsimd queue where nothing early waits
        load_x(0, split=True)
        HW2 = HPC * D
        nc.sync.dma_start(
            wqk[:, :, 0:HW2],
            wqk_d.ap()[:, 0:HW2].rearrange("(cc p) m -> p cc m", p=P_))
        nc.sync.dma_start(
            wqk[:, :, HW2:2 * HW2],
            wqk_d.ap()[:, HW2:2 * HW2].rearrange("(cc p) m -> p cc m", p=P_))
        nc.sync.dma_start(wv[:], wv_d.ap().rearrange("(cc p) m -> p cc m", p=P_))
        nc.gpsimd.dma_start(onesc[:], onesc_d.ap())
        nc.gpsimd.dma_start(maskv[:], mask_d.ap())
        nc.gpsimd.dma_start(es[:], es_d.ap())
        nc.gpsimd.dma_start(
            wproj[:], wproj_d.ap().rearrange("(co ci) m -> ci co m", ci=P_))
        nc.vector.tensor_copy(
            VO[:, :, :, D:P_],
            onesc[:, None, None, :].to_broadcast([P_, T // P_, HPC, D]))
        load_x(1)

        def qkv_ops(g):
            # deferred per-psum-group closures: 8 matmuls + 1 psum drain each
            tg0 = g * GW
            ops = []

            def qk_op(m, g=g, tg0=tg0):
                xg = xg_tiles[g]
                ps = psum.tile([P_, GW], F32, tag="qk", bufs=2,
                               name=f"qk{g}_{m}")
                for c in range(CCH):
                    nc.tensor.matmul(
                        ps[:], wqk[:, c, m * P_:(m + 1) * P_], xg[:, c, :],
                        start=(c == 0), stop=(c == CCH - 1))
                nc.vector.tensor_copy(QKT[:, m, tg0:tg0 + GW], ps[:])

            def v_op(tcl, g=g):
                xg = xg_tiles[g]
                tc_g = g * NTCG + tcl
                ps = psum.tile([P_, HPC * D], F32, tag="qk", bufs=2,
                               name=f"vps{g}_{tcl}")
                for c in range(CCH):
                    nc.tensor.matmul(
                        ps[:], xg[:, c, tcl * P_:(tcl + 1) * P_], wv[:, c, :],
                        start=(c == 0), stop=(c == CCH - 1))
                nc.vector.tensor_copy(
                    VO[:, tc_g, :, 0:D],
                    ps[:].rearrange("p (h d) -> p h d", h=HPC))

            for m in range(2 * NPAIR):
                ops.append(lambda m=m: qk_op(m))
            for tcl in range(NTCG):
                ops.append(lambda tcl=tcl: v_op(tcl))
            return ops

        def proj_ops(g, last=False):
            ops = []

            def tcl_op(tcl):
                ob = work.tile([P_, C], BF16, tag="ob", name=f"ob{tcl}")
                for nh in range(C // QB):
                    po = psum.tile([P_, QB], F32, tag="qk", bufs=2,
                                   name=f"po{tcl}_{nh}")
                    for cch in range(2):
                        nc.tensor.matmul(
                            po[:],
                            YT[:, cch, tcl * P_:(tcl + 1) * P_],
                            wproj[:, cch, nh * QB:(nh + 1) * QB],
                            start=(cch == 0), stop=(cch == 1))
                    # in the interleaved (non-last) case the scalar engine is
                    # busy with exp; at the tail it's idle, so split copies
                    if last and nh % 2 == 0:
                        nc.scalar.copy(ob[:, nh * QB:(nh + 1) * QB], po[:])
                    else:
                        nc.vector.tensor_copy(ob[:, nh * QB:(nh + 1) * QB],
                                              po[:])
                dq = (nc.sync if (last and tcl % 2 == 0) else nc.gpsimd)
                dq.dma_start(out_d.ap()[tcl * P_:(tcl + 1) * P_, :], ob[:])

            for tcl in range(g * NTCG, (g + 1) * NTCG):
                ops.append(lambda tcl=tcl: tcl_op(tcl))
            return ops

        for op in qkv_ops(0):
            op()

        for g in range(NG):
            tg0 = g * GW
            if g + 2 < NG:
                load_x(g + 2)
            fill = []
            if g + 1 < NG:
                fill.extend(qkv_ops(g + 1))
            if g >= 1:
                fill.extend(proj_ops(g - 1))
            fill.reverse()   # pop() serves in original order

            kmax = (g + 1) * NTCG
            kdiag = g * NTCG
            # spread deferred work across the group's chunk iterations so the
            # PE stays fed through the normalize/proj boundary at group end
            n_iters = kmax * NPAIR - (1 if g == 0 else 0)
            stride = max(1, n_iters // (len(fill) + 1)) if fill else 1
            it_ctr = 0
            for p in range(NPAIR):
                Y = [psum.tile([P_, QB], F32, tag=f"Y{e}",
                               name=f"Y{g}_{p}_{e}")[:, :GW]
                     for e in range(2)]
                pend = []

                def av(kc, Pt, p=p, Y=Y, kmax=kmax):
                    for e in range(2):
                        h = 2 * p + e
                        nc.tensor.matmul(
                            Y[e][:], VO[:, kc, h, :],
                            Pt[:, e * GW:(e + 1) * GW],
                            start=(kc == 0), stop=(kc == kmax - 1))

                for kc in range(kmax):
                    v = kc - kdiag
                    off = P_ * v if kc >= kdiag else 0
                    S = psum.tile([P_, 2 * GW], F32, tag="S", bufs=2,
                                  name=f"S{g}_{p}_{kc}")
                    Pt = work.tile([P_, 2 * GW], BF16, tag="P", bufs=4,
                                   name=f"Pt{g}_{p}_{kc}")
                    for e in range(2):
                        rows = slice(D * e, D * e + D)
                        nc.tensor.matmul(
                            S[:, e * GW + off:(e + 1) * GW],
                            QKT[rows, 2 + p, kc * P_:(kc + 1) * P_],
                            QKT[rows, p, tg0 + off:tg0 + GW],
                            start=True, stop=True)
                    Se = S[:].rearrange("p (e w) -> p e w", e=2)
                    Pe = Pt[:].rearrange("p (e w) -> p e w", e=2)
                    if off == 0:
                        nc.scalar.activation(
                            Pt[:], S[:], mybir.ActivationFunctionType.Exp,
                            scale=float(scale))
                    else:
                        nc.scalar.activation(
                            Pe[:, :, off:GW], Se[:, :, off:GW],
                            mybir.ActivationFunctionType.Exp,
                            scale=float(scale))
                    if kc >= kdiag:
                        nc.vector.tensor_tensor(
                            Pe[:, :, off:off + P_], Pe[:, :, off:off + P_],
                            maskv[:, None, :].to_broadcast([P_, 2, P_]),
                            mybir.AluOpType.mult)
                        if off:
                            nc.vector.memset(Pe[:, :, 0:off], 0)
                    # software pipeline: AV runs two chunks behind its exp,
                    # with one deferred qkv/proj psum-group interleaved so
                    # the PE never waits on the activation engine
                    if len(pend) >= 2:
                        av(*pend.pop(0))
                    if fill and (g > 0 or kc >= 1):
                        it_ctr += 1
                        if it_ctr % stride == 0:
                            fill.pop()()
                    pend.append((kc, Pt))
                while pend:
                    av(*pend.pop(0))

                for e in range(2):
                    h = 2 * p + e
                    scrA = work.tile([P_, GW], F32, tag="scrA",
                                     name=f"scrA{g}_{p}_{e}")
                    scrB = work.tile([P_, GW], F32, tag="scrB",
                                     name=f"scrB{g}_{p}_{e}")
                    # denom += exp(sink); fast approx reciprocal (base-0
                    # partitions only); cross-base mult needs one PSUM input
                    nc.vector.tensor_scalar_add(
                        scrA[0:D, :], Y[e][D:P_, :], es[0:D, h:h + 1])
                    nc.vector.reciprocal_approx_fast(
                        scrB[0:D, :], scrA[0:D, :])
                    nc.vector.tensor_tensor(
                        YT[D * e:D * e + D, p, tg0:tg0 + GW], Y[e][0:D, :],
                        scrB[0:D, :], mybir.AluOpType.mult)
            while fill:
                fill.pop()()
        for op in proj_ops(NG - 1, last=True):
            op()

    nc.compile()
    return nc


def _make_core_inputs(x, w_qkv, w_proj, sink_logit, core):
    b, g = core // 4, core % 4
    h0 = g * HPC
    HD = H * D

    xt = _bf16(np.asarray(x[b], dtype=np.float32).T)
    wq = w_qkv[:, h0 * D:(h0 + HPC) * D]
    wk = w_qkv[:, HD + h0 * D: HD + (h0 + HPC) * D]
    wvv = w_qkv[:, 2 * HD + h0 * D: 2 * HD + (h0 + HPC) * D]
    wqk = _bf16(np.concatenate([wq, wk], axis=1))
    wv = _bf16(wvv)
    wproj = _bf16(w_proj[h0 * D:(h0 + HPC) * D, :])

    es = np.zeros((P_, HPC), np.float32)
    for hh in range(HPC):
        es[:, hh] = np.exp(
            np.asarray(sink_logit[h0 + hh], dtype=np.float64)).astype(np.float32)

    mask = np.zeros((P_, P_), np.float32)
    for k in range(P_):
        mask[k, k:] = 1.0

    return {
        "xt": xt, "wqk": wqk, "wv": wv, "wproj": wproj, "essc": es,
        "onesc": np.ones((P_, D), ml_dtypes.bfloat16),
        "mask": mask.astype(ml_dtypes.bfloat16),
    }


_CACHE = {}


def _get_runner():
    """Build (once) the bass program and the jitted SPMD callable."""
    if "fn" in _CACHE:
        return _CACHE["fn"], _CACHE["meta"]

    import jax
    from jax.experimental.shard_map import shard_map
    from jax.sharding import Mesh, NamedSharding, PartitionSpec

    import concourse.mybir as mybir
    from concourse.bass2jax import (_bass_exec_p, install_neuronx_cc_hook,
                                    partition_id_tensor)

    nc = _build_bass()
    install_neuronx_cc_hook()
    pid_name = nc.partition_id_tensor.name if nc.partition_id_tensor else None

    in_names, out_names, out_avals, zero_outs = [], [], [], []
    for alloc in nc.m.functions[0].allocations:
        if not isinstance(alloc, mybir.MemoryLocationSet):
            continue
        name = alloc.memorylocations[0].name
        if alloc.kind == "ExternalInput":
            if name != pid_name:
                in_names.append(name)
        elif alloc.kind == "ExternalOutput":
            out_names.append(name)
            shape = tuple(alloc.tensor_shape)
            dtype = mybir.dt.np(alloc.dtype)
            out_avals.append(jax.core.ShapedArray(shape, dtype))
            zero_outs.append(np.zeros(shape, dtype))
    n_params, n_outs = len(in_names), len(out_avals)
    all_names = in_names + out_names
    if pid_name is not None:
        all_names = all_names + [pid_name]

    def _body(*args):
        operands = list(args)
        if pid_name is not None:
            operands.append(partition_id_tensor())
        outs = _bass_exec_p.bind(
            *operands,
            out_avals=tuple(out_avals),
            in_names=tuple(all_names),
            out_names=tuple(out_names),
            lowering_input_output_aliases=(),
            sim_require_finite=True,
            sim_require_nnan=True,
            nc=nc,
        )
        return tuple(outs)

    devices = jax.devices()[:N_CORES]
    mesh = Mesh(np.asarray(devices), ("core",))
    spec = PartitionSpec("core")
    sharding = NamedSharding(mesh, spec)
    fn = jax.jit(
        shard_map(_body, mesh=mesh, in_specs=(spec,) * (n_params + n_outs),
                  out_specs=(spec,) * n_outs, check_rep=False),
        keep_unused=True)

    zeros_dev = [jax.device_put(
        np.zeros((N_CORES * z.shape[0], *z.shape[1:]), z.dtype), sharding)
        for z in zero_outs]

    meta = dict(in_names=in_names, out_names=out_names, out_avals=out_avals,
                sharding=sharding, zeros_dev=zeros_dev, jax=jax)
    _CACHE["fn"] = fn
    _CACHE["meta"] = meta
    return fn, meta


def kernel(x, w_qkv, w_proj, sink_logit):
    x = np.asarray(x, dtype=np.float32)
    w_qkv = np.asarray(w_qkv, dtype=np.float32)
    w_proj = np.asarray(w_proj, dtype=np.float32)
    sink_logit = np.asarray(sink_logit, dtype=np.float32)

    fn, meta = _get_runner()
    jax = meta["jax"]

    in_maps = [_make_core_inputs(x, w_qkv, w_proj, sink_logit, core)
               for core in range(N_CORES)]
    concat_in = [
        jax.device_put(
            np.concatenate([in_maps[c][nm] for c in range(N_CORES)], axis=0),
            meta["sharding"])
        for nm in meta["in_names"]]

    out_arrs = fn(*concat_in, *meta["zeros_dev"])
    jax.block_until_ready(out_arrs)

    i_out = meta["out_names"].index("out")
    per_core = np.asarray(out_arrs[i_out]).reshape(N_CORES, T, C)

    out = np.zeros((B, T, C), np.float64)
    for core in range(N_CORES):
        out[core // 4] += per_core[core].astype(np.float64)
    return out.astype(np.float32)


# revision 30
# speedup vs baseline: 1.3454x; 1.0314x over previous
"""Causal self-attention with sink logit on 8 Trainium2 NeuronCores.

nn_CausalSelfAttention: B=2, T=2048, C=1024, H=16, D=64.
    qkv = x @ w_qkv; per-head causal attention with a per-head sink logit in
    the softmax denominator; out = y @ w_proj.

Sharding: 8 cores = 2 batches x 4 head-groups (data-parallel over B,
tensor-parallel over heads). Each core computes its batch's qkv projection
restricted to its 4 heads, flash-style causal attention (S^T layout,
denominator via an appended ones-block in the V matmul), and the partial
output projection against its w_proj row-slice. All matmul operands are
bf16 (1 cycle/row on the PE vs 2 for fp32r; fp32 PSUM accumulation), the
diagonal blocks compute only the un-masked column suffix, the per-head sink
is added to the denominator on the vector engine right before the fast
approximate DVE reciprocal, and the whole thing is software-pipelined: each
attention chunk's AV matmul issues one iteration behind its exp, with the
next group's qkv and the previous group's output-projection matmuls
interleaved between chunks so the PE never waits on the activation engine.
The host sums the 4 per-head-group bf16 partials per batch in float64 (the
"all-reduce after c_proj", done host-side since the full output is
assembled host-side anyway).

kernel(**inputs) takes the FULL unsharded inputs and returns the FULL output.
"""
from contextlib import ExitStack

import numpy as np
import ml_dtypes

F32 = None
BF16 = None

P_ = 128          # partitions
QB = 512          # psum bank width (fp32)
D = 64            # head dim
HPC = 4           # heads per core
NPAIR = 2
B, T, C, H = 2, 2048, 1024, 16
N_CORES = 8


def _bf16(x):
    return np.ascontiguousarray(np.asarray(x, dtype=np.float32)).astype(
        ml_dtypes.bfloat16)


def _build_bass():
    import concourse.mybir as mybir
    import concourse.tile as tile
    from concourse import bacc

    global F32, BF16
    F32 = mybir.dt.float32
    BF16 = mybir.dt.bfloat16

    CCH = C // P_             # C chunks
    GW = min(QB, T // 2)      # q/t group width
    NG = T // GW              # groups
    NTCG = GW // P_           # t-chunks per group
    scale = 1.0 / np.sqrt(D)

    nc = bacc.Bacc("TRN2", target_bir_lowering=False, debug=False,
                   num_devices=N_CORES)

    xt_d = nc.dram_tensor("xt", [C, T], BF16, kind="ExternalInput")
    wqk_d = nc.dram_tensor("wqk", [C, 2 * HPC * D], BF16, kind="ExternalInput")
    wv_d = nc.dram_tensor("wv", [C, HPC * D], BF16, kind="ExternalInput")
    wproj_d = nc.dram_tensor("wproj", [HPC * D, C], BF16, kind="ExternalInput")
    es_d = nc.dram_tensor("essc", [P_, HPC], F32, kind="ExternalInput")
    onesc_d = nc.dram_tensor("onesc", [P_, D], BF16, kind="ExternalInput")
    mask_d = nc.dram_tensor("mask", [P_, P_], BF16, kind="ExternalInput")
    out_d = nc.dram_tensor("out", [T, C], BF16, kind="ExternalOutput")

    with tile.TileContext(nc) as tc, ExitStack() as ctx:
        pool = ctx.enter_context(tc.tile_pool(name="pool", bufs=1))
        xt_pool = ctx.enter_context(tc.tile_pool(name="xt", bufs=3))
        work = ctx.enter_context(tc.tile_pool(name="work", bufs=2))
        psum = ctx.enter_context(tc.tile_pool(name="ps", bufs=1, space="PSUM"))

        es = pool.tile([P_, HPC], F32, tag="es")
        onesc = pool.tile([P_, D], BF16, tag="onesc")
        maskv = pool.tile([P_, P_], BF16, tag="maskv")
        wqk = pool.tile([P_, CCH, 2 * HPC * D], BF16, tag="wqk")
        wv = pool.tile([P_, CCH, HPC * D], BF16, tag="wv")
        wproj = pool.tile([P_, 2, C], BF16, tag="wproj")
        QKT = pool.tile([P_, 2 * NPAIR, T], BF16, tag="qkt")
        VO = pool.tile([P_, T // P_, HPC, P_], BF16, tag="vo")
        YT = pool.tile([P_, NPAIR, T], BF16, tag="yt")

        xg_tiles = {}
        xt_re = xt_d.ap().rearrange("(cc p) t -> p cc t", p=P_)

        def load_x(g, split=False):
            t = xt_pool.tile([P_, CCH, GW], BF16, tag="xt", name=f"x{g}")
            if split:
                nc.scalar.dma_start(
                    t[:, 0:CCH // 2, :],
                    xt_re[:, 0:CCH // 2, g * GW:(g + 1) * GW])
                nc.scalar.dma_start(
                    t[:, CCH // 2:CCH, :],
                    xt_re[:, CCH // 2:CCH, g * GW:(g + 1) * GW])
            else:
                nc.scalar.dma_start(t[:], xt_re[:, :, g * GW:(g + 1) * GW])
            xg_tiles[g] = t

        # startup: halve the first x/wqk transfers so the first qk matmul
        # chain only waits on ~512KB per queue, not the full 2MB; small/late
        # tensors (es has a pathologically slow 16B-per-partition descriptor)
        # go on the gpsimd queue where nothing early waits
        load_x(0, split=True)
        HW2 = HPC * D
        nc.sync.dma_start(
            wqk[:, :, 0:HW2],
            wqk_d.ap()[:, 0:HW2].rearrange("(cc p) m -> p cc m", p=P_))
        nc.sync.dma_start(
            wqk[:, :, HW2:2 * HW2],
            wqk_d.ap()[:, HW2:2 * HW2].rearrange("(cc p) m -> p cc m", p=P_))
        nc.sync.dma_start(wv[:], wv_d.ap().rearrange("(cc p) m -> p cc m", p=P_))
        nc.gpsimd.dma_start(onesc[:], onesc_d.ap())
        nc.gpsimd.dma_start(maskv[:], mask_d.ap())
        nc.gpsimd.dma_start(es[:], es_d.ap())
        nc.gpsimd.dma_start(
            wproj[:], wproj_d.ap().rearrange("(co ci) m -> ci co m", ci=P_))
        nc.vector.tensor_copy(
            VO[:, :, :, D:P_],
            onesc[:, None, None, :].to_broadcast([P_, T // P_, HPC, D]))
        load_x(1)

        def qkv_ops(g):
            # deferred per-psum-group closures: 8 matmuls + 1 psum drain each
            tg0 = g * GW
            ops = []

            def qk_op(m, g=g, tg0=tg0):
                xg = xg_tiles[g]
                ps = psum.tile([P_, GW], F32, tag="qk", bufs=2,
                               name=f"qk{g}_{m}")
                for c in range(CCH):
                    nc.tensor.matmul(
                        ps[:], wqk[:, c, m * P_:(m + 1) * P_], xg[:, c, :],
                        start=(c == 0), stop=(c == CCH - 1))
                nc.vector.tensor_copy(QKT[:, m, tg0:tg0 + GW], ps[:])

            def v_op(tcl, g=g):
                xg = xg_tiles[g]
                tc_g = g * NTCG + tcl
                ps = psum.tile([P_, HPC * D], F32, tag="qk", bufs=2,
                               name=f"vps{g}_{tcl}")
                for c in range(CCH):
                    nc.tensor.matmul(
                        ps[:], xg[:, c, tcl * P_:(tcl + 1) * P_], wv[:, c, :],
                        start=(c == 0), stop=(c == CCH - 1))
                nc.vector.tensor_copy(
                    VO[:, tc_g, :, 0:D],
                    ps[:].rearrange("p (h d) -> p h d", h=HPC))

            for m in range(2 * NPAIR):
                ops.append(lambda m=m: qk_op(m))
            for tcl in range(NTCG):
                ops.append(lambda tcl=tcl: v_op(tcl))
            return ops

        def proj_ops(g, last=False):
            ops = []

            def tcl_op(tcl):
                ob = work.tile([P_, C], BF16, tag="ob", name=f"ob{tcl}")
                for nh in range(C // QB):
                    po = psum.tile([P_, QB], F32, tag="qk", bufs=2,
                                   name=f"po{tcl}_{nh}")
                    for cch in range(2):
                        nc.tensor.matmul(
                            po[:],
                            YT[:, cch, tcl * P_:(tcl + 1) * P_],
                            wproj[:, cch, nh * QB:(nh + 1) * QB],
                            start=(cch == 0), stop=(cch == 1))
                    # in the interleaved (non-last) case the scalar engine is
                    # busy with exp; at the tail it's idle, so split copies
                    # and stream each half out as soon as it's copied
                    if last and nh % 2 == 0:
                        nc.scalar.copy(ob[:, nh * QB:(nh + 1) * QB], po[:])
                    else:
                        nc.vector.tensor_copy(ob[:, nh * QB:(nh + 1) * QB],
                                              po[:])
                    if last:
                        dq = nc.sync if nh % 2 == 0 else nc.gpsimd
                        dq.dma_start(
                            out_d.ap()[tcl * P_:(tcl + 1) * P_,
                                       nh * QB:(nh + 1) * QB],
                            ob[:, nh * QB:(nh + 1) * QB])
                if not last:
                    nc.gpsimd.dma_start(
                        out_d.ap()[tcl * P_:(tcl + 1) * P_, :], ob[:])

            for tcl in range(g * NTCG, (g + 1) * NTCG):
                ops.append(lambda tcl=tcl: tcl_op(tcl))
            return ops

        for op in qkv_ops(0):
            op()

        for g in range(NG):
            tg0 = g * GW
            if g + 2 < NG:
                load_x(g + 2)
            fill = []
            if g + 1 < NG:
                fill.extend(qkv_ops(g + 1))
            if g >= 1:
                fill.extend(proj_ops(g - 1))
            fill.reverse()   # pop() serves in original order

            kmax = (g + 1) * NTCG
            kdiag = g * NTCG
            # spread deferred work across the group's chunk iterations so the
            # PE stays fed through the normalize/proj boundary at group end
            n_iters = kmax * NPAIR - (1 if g == 0 else 0)
            stride = max(1, n_iters // max(len(fill), 1)) if fill else 1
            it_ctr = 0
            for p in range(NPAIR):
                Y = [psum.tile([P_, QB], F32, tag=f"Y{e}",
                               name=f"Y{g}_{p}_{e}")[:, :GW]
                     for e in range(2)]
                pend = []

                def av(kc, Pt, p=p, Y=Y, kmax=kmax):
                    for e in range(2):
                        h = 2 * p + e
                        nc.tensor.matmul(
                            Y[e][:], VO[:, kc, h, :],
                            Pt[:, e * GW:(e + 1) * GW],
                            start=(kc == 0), stop=(kc == kmax - 1))

                for kc in range(kmax):
                    v = kc - kdiag
                    off = P_ * v if kc >= kdiag else 0
                    S = psum.tile([P_, 2 * GW], F32, tag="S", bufs=2,
                                  name=f"S{g}_{p}_{kc}")
                    Pt = work.tile([P_, 2 * GW], BF16, tag="P", bufs=4,
                                   name=f"Pt{g}_{p}_{kc}")
                    for e in range(2):
                        rows = slice(D * e, D * e + D)
                        nc.tensor.matmul(
                            S[:, e * GW + off:(e + 1) * GW],
                            QKT[rows, 2 + p, kc * P_:(kc + 1) * P_],
                            QKT[rows, p, tg0 + off:tg0 + GW],
                            start=True, stop=True)
                    Se = S[:].rearrange("p (e w) -> p e w", e=2)
                    Pe = Pt[:].rearrange("p (e w) -> p e w", e=2)
                    if off == 0:
                        nc.scalar.activation(
                            Pt[:], S[:], mybir.ActivationFunctionType.Exp,
                            scale=float(scale))
                    else:
                        nc.scalar.activation(
                            Pe[:, :, off:GW], Se[:, :, off:GW],
                            mybir.ActivationFunctionType.Exp,
                            scale=float(scale))
                    if kc >= kdiag:
                        nc.vector.tensor_tensor(
                            Pe[:, :, off:off + P_], Pe[:, :, off:off + P_],
                            maskv[:, None, :].to_broadcast([P_, 2, P_]),
                            mybir.AluOpType.mult)
                        if off:
                            nc.vector.memset(Pe[:, :, 0:off], 0)
                    # software pipeline: AV runs two chunks behind its exp,
                    # with one deferred qkv/proj psum-group interleaved so
                    # the PE never waits on the activation engine
                    if len(pend) >= 2:
                        av(*pend.pop(0))
                    if fill and (g > 0 or kc >= 1):
                        it_ctr += 1
                        if it_ctr % stride == 0:
                            fill.pop()()
                    pend.append((kc, Pt))
                while pend:
                    av(*pend.pop(0))

                for e in range(2):
                    h = 2 * p + e
                    scrA = work.tile([P_, GW], F32, tag="scrA",
                                     name=f"scrA{g}_{p}_{e}")
                    scrB = work.tile([P_, GW], F32, tag="scrB",
                                     name=f"scrB{g}_{p}_{e}")
                    # denom += exp(sink); fast approx reciprocal (base-0
                    # partitions only); cross-base mult needs one PSUM input
                    nc.vector.tensor_scalar_add(
                        scrA[0:D, :], Y[e][D:P_, :], es[0:D, h:h + 1])
                    nc.vector.reciprocal_approx_fast(
                        scrB[0:D, :], scrA[0:D, :])
                    nc.vector.tensor_tensor(
                        YT[D * e:D * e + D, p, tg0:tg0 + GW], Y[e][0:D, :],
                        scrB[0:D, :], mybir.AluOpType.mult)
            while fill:
                fill.pop()()
        for op in proj_ops(NG - 1, last=True):
            op()

    nc.compile()
    return nc


def _make_core_inputs(x, w_qkv, w_proj, sink_logit, core):
    b, g = core // 4, core % 4
    h0 = g * HPC
    HD = H * D

    xt = _bf16(np.asarray(x[b], dtype=np.float32).T)
    wq = w_qkv[:, h0 * D:(h0 + HPC) * D]
    wk = w_qkv[:, HD + h0 * D: HD + (h0 + HPC) * D]
    wvv = w_qkv[:, 2 * HD + h0 * D: 2 * HD + (h0 + HPC) * D]
    wqk = _bf16(np.concatenate([wq, wk], axis=1))
    wv = _bf16(wvv)
    wproj = _bf16(w_proj[h0 * D:(h0 + HPC) * D, :])

    es = np.zeros((P_, HPC), np.float32)
    for hh in range(HPC):
        es[:, hh] = np.exp(
            np.asarray(sink_logit[h0 + hh], dtype=np.float64)).astype(np.float32)

    mask = np.zeros((P_, P_), np.float32)
    for k in range(P_):
        mask[k, k:] = 1.0

    return {
        "xt": xt, "wqk": wqk, "wv": wv, "wproj": wproj, "essc": es,
        "onesc": np.ones((P_, D), ml_dtypes.bfloat16),
        "mask": mask.astype(ml_dtypes.bfloat16),
    }


_CACHE = {}


def _get_runner():
    """Build (once) the bass program and the jitted SPMD callable."""
    if "fn" in _CACHE:
        return _CACHE["fn"], _CACHE["meta"]

    import jax
    from jax.experimental.shard_map import shard_map
    from jax.sharding import Mesh, NamedSharding, PartitionSpec

    import concourse.mybir as mybir
    from concourse.bass2jax import (_bass_exec_p, install_neuronx_cc_hook,
                                    partition_id_tensor)

    nc = _build_bass()
    install_neuronx_cc_hook()
    pid_name = nc.partition_id_tensor.name if nc.partition_id_tensor else None

    in_names, out_names, out_avals, zero_outs = [], [], [], []
    for alloc in nc.m.functions[0].allocations:
        if not isinstance(alloc, mybir.MemoryLocationSet):
            continue
        name = alloc.memorylocations[0].name
        if alloc.kind == "ExternalInput":
            if name != pid_name:
                in_names.append(name)
        elif alloc.kind == "ExternalOutput":
            out_names.append(name)
            shape = tuple(alloc.tensor_shape)
            dtype = mybir.dt.np(alloc.dtype)
            out_avals.append(jax.core.ShapedArray(shape, dtype))
            zero_outs.append(np.zeros(shape, dtype))
    n_params, n_outs = len(in_names), len(out_avals)
    all_names = in_names + out_names
    if pid_name is not None:
        all_names = all_names + [pid_name]

    def _body(*args):
        operands = list(args)
        if pid_name is not None:
            operands.append(partition_id_tensor())
        outs = _bass_exec_p.bind(
            *operands,
            out_avals=tuple(out_avals),
            in_names=tuple(all_names),
            out_names=tuple(out_names),
            lowering_input_output_aliases=(),
            sim_require_finite=True,
            sim_require_nnan=True,
            nc=nc,
        )
        return tuple(outs)

    devices = jax.devices()[:N_CORES]
    mesh = Mesh(np.asarray(devices), ("core",))
    spec = PartitionSpec("core")
    sharding = NamedSharding(mesh, spec)
    fn = jax.jit(
        shard_map(_body, mesh=mesh, in_specs=(spec,) * (n_params + n_outs),
                  out_specs=(spec,) * n_outs, check_rep=False),
        keep_unused=True)

    zeros_dev = [jax.device_put(
        np.zeros((N_CORES * z.shape[0], *z.shape[1:]), z.dtype), sharding)
        for z in zero_outs]

    meta = dict(in_names=in_names, out_names=out_names, out_avals=out_avals,
                sharding=sharding, zeros_dev=zeros_dev, jax=jax)
    _CACHE["fn"] = fn
    _CACHE["meta"] = meta
    return fn, meta


def kernel(x, w_qkv, w_proj, sink_logit):
    x = np.asarray(x, dtype=np.float32)
    w_qkv = np.asarray(w_qkv, dtype=np.float32)
    w_proj = np.asarray(w_proj, dtype=np.float32)
    sink_logit = np.asarray(sink_logit, dtype=np.float32)

    fn, meta = _get_runner()
    jax = meta["jax"]

    in_maps = [_make_core_inputs(x, w_qkv, w_proj, sink_logit, core)
               for core in range(N_CORES)]
    concat_in = [
        jax.device_put(
            np.concatenate([in_maps[c][nm] for c in range(N_CORES)], axis=0),
            meta["sharding"])
        for nm in meta["in_names"]]

    out_arrs = fn(*concat_in, *meta["zeros_dev"])
    jax.block_until_ready(out_arrs)

    i_out = meta["out_names"].index("out")
    per_core = np.asarray(out_arrs[i_out]).reshape(N_CORES, T, C)

    out = np.zeros((B, T, C), np.float64)
    for core in range(N_CORES):
        out[core // 4] += per_core[core].astype(np.float64)
    return out.astype(np.float32)
